# revision 6
# baseline (speedup 1.0000x reference)
"""2-layer GAT (PyG-style) on TRN2, 8 NeuronCores — single-NEFF version.

Nodes sorted by in-degree (self-loops excluded) are dealt round-robin across
8 cores into 49 tiles/core of 128 nodes. Both layers' tables live in a
global (core, tile, partition) row order split in two half-regions so each
half can be AllGathered early; per-edge source rows are fetched with
dma_gather in slot-major order (dst node == partition), striped across the
4 SWDGE queues per tile-group. Self-loop rows and per-tile a_d come from the
core's local shard at static offsets (no gather).

dense1 is sharded: each core computes only its own 49 tiles, then two
AllGathers assemble the full layer-1 table (first half overlaps the rest of
dense1). Layer-2 table is assembled the same way, overlapping layer-1
compute. One index table serves both layers (same global row mapping).

Aggregation per tile: attention weights are normalized (alpha = w / sum w)
BEFORE the multiply, the gathered rows are scaled in place on DVE (feature
order is pair-duplicated per head so every operand streams at innermost
stride 1), then a 2-level pairwise bf16 tree halves the slot count twice and
the remaining ~d/4 slots + the self row are summed into f32 PSUM with
identity-lhsT matmuls on the TensorEngine.

Feature order inside rows: position i = cp*6 + h*2 + par holds feature
f = h*64 + cp*2 + par  (pairs of features share a head => duplicated-weight
vectors are contiguous pairs => DVE 2x mode).
"""
import numpy as np
import ml_dtypes

N = 50000
E = 800000
IN = 128
HID = 64
HEADS = 3
OUT = 64
NCORE = 8
P = 128
T = 49                              # tiles per core
CB = T * P + P                      # 6400 rows per core shard (incl pad tile)
NTAB = NCORE * CB                   # 51200
SENTROW = 51072                     # region-B core-7 pad row (global)
BASE = 32768
E1 = 256                            # L1 row elems (bf16) = 512B
E2 = 128                            # L2 row elems (bf16) = 256B
F1 = HEADS * HID                    # 192
NEG_SLOPE = 0.2
NQ = 4                              # SWDGE queues
GSZ = 3                             # tiles per gather group
XC = 7                              # dense1 tiles per chunk (49 = 7*7)
HALF_T = 25                         # tiles 0..24 -> AG region A, 25..48 -> B

bf16 = ml_dtypes.bfloat16

# (c,h)-interleaved feature order: row position i = c*3+h holds f = h*64+c
_POS = np.arange(F1)
PERM = (_POS % HEADS) * HID + (_POS // HEADS)
HPOS = _POS % HEADS                 # head of row position i


def _pack_idx(rows_flat):
    v = (rows_flat - BASE).astype(np.int16)
    cid = len(v) // 16
    block = v.reshape(cid, 16).T
    return np.tile(block, (8, 1))


def _g2row(c, t, p):
    t = np.asarray(t)
    return np.where(t < HALF_T,
                    c * 3200 + t * 128 + p,
                    25600 + c * 3200 + (t - HALF_T) * 128 + p)


def preprocess(edge_index):
    src = edge_index[0].astype(np.int64)
    dst = edge_index[1].astype(np.int64)
    deg = np.bincount(dst, minlength=N)            # in-degree, no self loops
    order = np.argsort(-deg, kind="stable")
    eorder = np.argsort(dst, kind="stable")
    esrc = src[eorder]
    eptr = np.zeros(N + 1, np.int64)
    eptr[1:] = np.cumsum(deg)

    n127 = T * NCORE
    main = order[:N - n127]
    tail = order[N - n127:]
    nid = np.full((NCORE, T, P), -1, np.int64)
    MP = P - 1
    for t in range(T):
        blk = main[t * NCORE * MP: (t + 1) * NCORE * MP]
        for c in range(NCORE):
            sl = blk[c::NCORE]
            nid[c, t, :len(sl)] = sl
        tb = tail[t * NCORE: (t + 1) * NCORE]
        for c in range(NCORE):
            if c < len(tb):
                nid[c, t, P - 1] = tb[c]

    posc = np.zeros(N, np.int64)
    post = np.zeros(N, np.int64)
    posp = np.zeros(N, np.int64)
    for c in range(NCORE):
        for t in range(T):
            nn = nid[c, t]
            v = nn >= 0
            posc[nn[v]] = c
            post[nn[v]] = t
            posp[nn[v]] = np.nonzero(v)[0]

    dmax = np.zeros(T, np.int64)
    d127 = np.zeros(T, np.int64)
    for t in range(T):
        for c in range(NCORE):
            if nid[c, t, 0] >= 0:
                dmax[t] = max(dmax[t], deg[nid[c, t, 0]])
            if nid[c, t, P - 1] >= 0:
                d127[t] = max(d127[t], deg[nid[c, t, P - 1]])
    d_t = np.maximum(dmax, 1)

    ng = (T + GSZ - 1) // GSZ
    while True:
        bins = [[] for _ in range(ng)]
        load = np.zeros(ng, np.int64)
        cnt = np.zeros(ng, np.int64)
        for t in np.argsort(-d_t, kind="stable"):
            elig = np.nonzero(cnt < GSZ)[0]
            b = elig[np.argmin(load[elig])]
            bins[b].append(int(t))
            load[b] += d_t[t]
            cnt[b] += 1
        groups = [sorted(b) for b in bins]
        stripes = []
        bnd_local = {t: set() for t in range(T)}
        for g, mem in enumerate(groups):
            S = int(sum(d_t[t] for t in mem))
            bs = sorted(set(max(1, round(S * q / NQ)) for q in range(1, NQ + 1)))
            st = []
            s0 = 0
            for b in bs:
                if b > s0:
                    st.append((s0, b))
                    s0 = b
            stripes.append(st)
            offs = np.cumsum([0] + [int(d_t[t]) for t in mem])
            for (a, b) in st:
                e = b - 1
                mi = int(np.searchsorted(offs, e, side="right")) - 1
                bnd_local[mem[mi]].add(int(e - offs[mi]))
        ok = True
        for t in range(T):
            avail = int(d_t[t]) - len(bnd_local[t])
            if d127[t] > avail:
                d_t[t] += d127[t] - avail
                ok = False
        if ok:
            break

    goff = []
    for mem in groups:
        goff.append(np.cumsum([0] + [int(d_t[t]) for t in mem]))
    CE = int(d_t.sum()) * 8

    idxg = np.zeros((NCORE, P, CE), np.int16)
    grow = _g2row(posc, post, posp)
    p127s = {}
    for t in range(T):
        d = int(d_t[t])
        p127s[t] = [s for s in range(d) if s not in bnd_local[t]]
    for c in range(NCORE):
        blocks = []
        for g, mem in enumerate(groups):
            S = int(goff[g][-1])
            rr = np.full((S, P), SENTROW, np.int64)
            for mi, t in enumerate(mem):
                off = int(goff[g][mi])
                for p in range(P):
                    n = nid[c, t, p]
                    if n < 0:
                        continue
                    k = int(deg[n])
                    ee = esrc[eptr[n]:eptr[n] + k]
                    sl = (np.array(p127s[t][:k], np.int64) if p == P - 1
                          else np.arange(k))
                    rr[off + sl, p] = grow[ee]
            blocks.append(_pack_idx(rr.reshape(-1)))
        idxg[c] = np.concatenate(blocks, axis=1)

    return dict(order=order, nid=nid, d_t=d_t, deg=deg, esrc=esrc, eptr=eptr,
                groups=groups, stripes=stripes, goff=goff, idxg=idxg,
                posc=posc, post=post, posp=posp, p127s=p127s)


def host_weights(x, W1, att_src1, att_dst1, b1, W2, att_src2, att_dst2, b2,
                 nid):
    W1s = np.stack([W1[:, h * HID:(h + 1) * HID] @ att_src1[h]
                    for h in range(HEADS)], 1)
    W1d = np.stack([W1[:, h * HID:(h + 1) * HID] @ att_dst1[h]
                    for h in range(HEADS)], 1)
    W1e = np.concatenate([W1[:, PERM], W1s, W1d], axis=1)        # [128,198]
    W2e = np.concatenate([W2, (W2 @ att_src2[0])[:, None],
                          (W2 @ att_dst2[0])[:, None]], 1)[PERM, :]
    xTs = []
    for c in range(NCORE):
        Xc = np.zeros((IN, T * P), np.float32)
        for t in range(T):
            nn = nid[c, t]
            v = nn >= 0
            Xc[:, t * P + np.nonzero(v)[0]] = x[nn[v]].T
        xTs.append(Xc.astype(bf16))
    sent1 = np.zeros(E1, bf16)
    sent1.view(np.float32)[96:99] = -1e30
    sent2 = np.zeros(E2, bf16)
    sent2.view(np.float32)[32] = -1e30
    b1i = b1[PERM].astype(bf16)
    neg1 = np.full((P, F1), -1.0, bf16)
    return dict(xTs=xTs, W1e=W1e.astype(bf16),
                W2e1=W2e[:128].astype(bf16), W2e2=W2e[128:].astype(bf16),
                sent1=sent1.reshape(1, E1), sent2=sent2.reshape(1, E2),
                b1i=np.tile(b1i, (P, 1)), neg1=neg1)


# ---------------------------------------------------------------- emulation
def _bf(a):
    return a.astype(bf16).astype(np.float32)


def emulate(x, edge_index, W1, att_src1, att_dst1, b1, W2, att_src2, att_dst2,
            b2):
    pre = preprocess(edge_index)
    hw = host_weights(x, W1, att_src1, att_dst1, b1, W2, att_src2, att_dst2,
                      b2, pre["nid"])
    nid, d_t = pre["nid"], pre["d_t"]
    deg, esrc, eptr = pre["deg"], pre["esrc"], pre["eptr"]
    p127s = pre["p127s"]

    W1ef = hw["W1e"].astype(np.float32)
    b1f = hw["b1i"].astype(np.float32)[0]
    W2e1f = hw["W2e1"].astype(np.float32)
    W2e2f = hw["W2e2"].astype(np.float32)

    tab_h = np.zeros((N, F1), np.float32)
    tab_as = np.zeros((N, HEADS), np.float32)
    tab_ad = np.zeros((N, HEADS), np.float32)
    for c in range(NCORE):
        for t in range(T):
            nn = nid[c, t]
            v = nn >= 0
            H = _bf(x[nn[v]]) @ W1ef
            tab_h[nn[v]] = _bf(H[:, :F1])
            tab_as[nn[v]] = H[:, F1:F1 + 3]
            tab_ad[nn[v]] = H[:, F1 + 3:F1 + 6]

    tab2_h = np.zeros((N, OUT), np.float32)
    tab2_as = np.zeros((N, 1), np.float32)
    tab2_ad = np.zeros((N, 1), np.float32)

    def gat_layer(th, tas, tad, nf, layer):
        H_ = tas.shape[1]
        outs = np.zeros((NCORE, T, P, nf), np.float32)
        for c in range(NCORE):
            for t in range(T):
                d = int(d_t[t])
                rows = np.full((P, d), -1, np.int64)
                for p in range(P):
                    n = nid[c, t, p]
                    if n < 0:
                        continue
                    k = int(deg[n])
                    sl = p127s[t][:k] if p == P - 1 else list(range(k))
                    rows[p, sl] = esrc[eptr[n]:eptr[n] + k]
                pad = rows < 0
                rr = np.where(pad, 0, rows)
                g_h = np.where(pad[:, :, None], 0.0, th[rr])
                g_as = np.where(pad[:, :, None], -1e30, tas[rr])
                own = nid[c, t]
                ov = own >= 0
                oh = np.where(ov[:, None], th[np.where(ov, own, 0)], 0.0)
                oas = np.where(ov[:, None], tas[np.where(ov, own, 0)], 0.0)
                oad = np.where(ov[:, None], tad[np.where(ov, own, 0)], 0.0)
                eE = g_as + oad[:, None, :]
                eE = np.maximum(eE, NEG_SLOPE * eE)
                wE = np.exp(eE)
                eS = oas + oad
                eS = np.maximum(eS, NEG_SLOPE * eS)
                wS = np.exp(eS)
                s = wE.sum(axis=1) + wS
                r = 1.0 / np.maximum(s, 1e-30)
                aE = _bf(wE * r[:, None, :])
                aS = _bf(wS * r)
                if layer == 1:
                    wexp = aE[:, :, HPOS]
                    sexp = aS[:, HPOS]
                else:
                    wexp = np.repeat(aE, nf, axis=2)
                    sexp = np.repeat(aS, nf, axis=1)
                prod = _bf(g_h * wexp)
                sprod = _bf(oh * sexp)
                # 2-level bf16 tree, then f32 sum + self
                vals = [prod[:, j] for j in range(d)]
                for _ in range(2):
                    if len(vals) < 2:
                        break
                    nxt = [_bf(vals[j] + vals[j + 1])
                           for j in range(0, len(vals) - 1, 2)]
                    if len(vals) % 2:
                        nxt.append(vals[-1])
                    vals = nxt
                U = sprod.astype(np.float32)
                for vv in vals:
                    U = U + vv
                if layer == 1:
                    h1 = _bf(U)
                    h1 = _bf(h1 + b1f)
                    tmin = _bf(np.minimum(h1, 0))
                    texp = _bf(np.exp(tmin))
                    h1 = _bf(_bf(np.maximum(h1, 0) + texp) - 1.0)
                    outs[c, t] = h1
                else:
                    outs[c, t] = U
        return outs

    h1 = gat_layer(tab_h, tab_as, tab_ad, F1, 1)
    for c in range(NCORE):
        for t in range(T):
            o2 = _bf(h1[c, t]) @ np.concatenate([W2e1f, W2e2f], 0)
            nn = nid[c, t]
            v = nn >= 0
            tab2_h[nn[v]] = _bf(o2[v, :OUT])
            tab2_as[nn[v], 0] = o2[v, OUT]
            tab2_ad[nn[v], 0] = o2[v, OUT + 1]
    out = gat_layer(tab2_h, tab2_as, tab2_ad, OUT, 2)
    res = np.zeros((N, OUT), np.float32)
    for c in range(NCORE):
        for t in range(T):
            nn = nid[c, t]
            v = nn >= 0
            res[nn[v]] = out[c, t][v] + b2[None, :]
    return res


# ---------------------------------------------------------------- bass build
def _build(pre, pay1_bufs=4, pay2_bufs=4):
    import concourse.bacc as bacc
    import concourse.mybir as mybir
    import concourse.tile as tile
    from concourse.masks import make_identity

    d_t = pre["d_t"]
    groups, stripes, goff = pre["groups"], pre["stripes"], pre["goff"]
    CE = int(d_t.sum()) * 8
    MAXS = max(int(goff[g][-1]) for g in range(len(groups)))
    T1 = (MAXS + 1) // 2
    T2 = (T1 + 1) // 2

    dt = mybir.dt
    nc = bacc.Bacc(num_devices=NCORE, num_swdge_queues=NQ)
    xT = nc.dram_tensor("xT", [IN, T * P], dt.bfloat16, kind="ExternalInput")
    W1e = nc.dram_tensor("W1e", [IN, 198], dt.bfloat16, kind="ExternalInput")
    W2e1 = nc.dram_tensor("W2e1", [128, 66], dt.bfloat16, kind="ExternalInput")
    W2e2 = nc.dram_tensor("W2e2", [64, 66], dt.bfloat16, kind="ExternalInput")
    b1i = nc.dram_tensor("b1i", [P, F1], dt.bfloat16, kind="ExternalInput")
    neg1 = nc.dram_tensor("neg1", [P, F1], dt.bfloat16, kind="ExternalInput")
    sent1 = nc.dram_tensor("sent1", [1, E1], dt.bfloat16, kind="ExternalInput")
    sent2 = nc.dram_tensor("sent2", [1, E2], dt.bfloat16, kind="ExternalInput")
    idxg = nc.dram_tensor("idxg", [P, CE], dt.int16, kind="ExternalInput")
    out2 = nc.dram_tensor("out2", [T * P, OUT], dt.float32, kind="ExternalOutput")
    shard1 = nc.dram_tensor("shard1", [CB, E1], dt.bfloat16)
    shard2 = nc.dram_tensor("shard2", [CB, E2], dt.bfloat16)
    ag1 = nc.dram_tensor("ag1", [NTAB, E1], dt.bfloat16, addr_space="Shared")
    ag2 = nc.dram_tensor("ag2", [NTAB, E2], dt.bfloat16, addr_space="Shared")
    RG = [list(range(NCORE))]
    HP = HALF_T * P                 # 3200

    with tile.TileContext(nc) as tc:
        with tc.tile_pool(name="const", bufs=1) as cp:
            w1_sb = cp.tile([IN, 198], dt.bfloat16)
            nc.sync.dma_start(out=w1_sb[:], in_=W1e[:, :])
            w2a_sb = cp.tile([128, 66], dt.bfloat16)
            nc.sync.dma_start(out=w2a_sb[:], in_=W2e1[:, :])
            w2b_sb = cp.tile([64, 66], dt.bfloat16)
            nc.sync.dma_start(out=w2b_sb[:], in_=W2e2[:, :])
            b1_sb = cp.tile([P, F1], dt.bfloat16)
            nc.sync.dma_start(out=b1_sb[:], in_=b1i[:, :])
            n1_sb = cp.tile([P, F1], dt.bfloat16)
            nc.sync.dma_start(out=n1_sb[:], in_=neg1[:, :])
            ide = cp.tile([P, P], dt.bfloat16)
            make_identity(nc, ide[:])
            ix_sb = cp.tile([P, CE], dt.int16)
            nc.sync.dma_start(out=ix_sb[:], in_=idxg[:, :])
            sent1_sb = cp.tile([1, E1], dt.bfloat16)
            nc.sync.dma_start(out=sent1_sb[:], in_=sent1[:, :])
            nc.sync.dma_start(out=shard1[T * P:T * P + 1, :], in_=sent1_sb[:])
            sent2_sb = cp.tile([1, E2], dt.bfloat16)
            nc.sync.dma_start(out=sent2_sb[:], in_=sent2[:, :])
            nc.sync.dma_start(out=shard2[T * P:T * P + 1, :], in_=sent2_sb[:])

            # ---------------- dense1 (own tiles only)
            with tc.tile_pool(name="xp", bufs=3) as xp, \
                 tc.tile_pool(name="rowp", bufs=3) as rowp, \
                 tc.tile_pool(name="psD", bufs=4, space="PSUM") as psD:
                for j in range(XC):
                    xch = xp.tile([P, XC * P], dt.bfloat16, tag="x")
                    nc.sync.dma_start(
                        out=xch[:], in_=xT[:, j * XC * P:(j + 1) * XC * P])
                    rt = rowp.tile([P, XC * E1], dt.bfloat16, tag="rt")
                    for k in range(XC):
                        pt = psD.tile([P, 198], dt.float32, tag="d1")
                        nc.tensor.matmul(pt[:], lhsT=xch[:, k * P:(k + 1) * P],
                                         rhs=w1_sb[:], start=True, stop=True)
                        nc.scalar.activation(rt[:, k * E1:k * E1 + F1],
                                             pt[:, :F1],
                                             mybir.ActivationFunctionType.Copy)
                        nc.vector.tensor_copy(
                            out=rt[:, k * E1 + F1:k * E1 + F1 + 12]
                                .bitcast(dt.float32),
                            in_=pt[:, F1:198])
                    nc.sync.dma_start(
                        out=shard1[j * XC * P:(j + 1) * XC * P, :]
                            .rearrange("(k p) e -> p k e", p=P),
                        in_=rt[:].rearrange("p (k e) -> p k e", e=E1))

            # ---------------- AllGather layer-1 table (two halves)
            nc.gpsimd.collective_compute(
                "AllGather", mybir.AluOpType.bypass, replica_groups=RG,
                ins=[shard1[0:HP, :].opt()],
                outs=[ag1[0:NCORE * HP, :].opt()])
            nc.gpsimd.collective_compute(
                "AllGather", mybir.AluOpType.bypass, replica_groups=RG,
                ins=[shard1[HP:CB, :].opt()],
                outs=[ag1[NCORE * HP:NTAB, :].opt()])

            def edge_phase(layer, in_lo, shard_own, EW, F, payp, ownp, wp,
                           trp, hp, psU, psB, psO, NH):
                for g, mem in enumerate(groups):
                    M = len(mem)
                    S = int(goff[g][-1])
                    off_cols = int(np.sum([int(goff[gg][-1])
                                           for gg in range(g)]))
                    own = ownp.tile([P, GSZ * EW], dt.bfloat16, tag="own")
                    for mi, t in enumerate(mem):
                        nc.scalar.dma_start(
                            out=own[:, mi * EW:(mi + 1) * EW],
                            in_=shard_own[t * P:(t + 1) * P, :])
                    pay = payp.tile([P, MAXS * EW], dt.bfloat16, tag="pay")
                    for q, (s0, s1) in enumerate(stripes[g]):
                        nc.gpsimd.dma_gather(
                            out_ap=pay[:, s0 * EW:s1 * EW]
                                .rearrange("p (s e) -> p s e", e=EW),
                            in_ap=in_lo,
                            idxs_ap=ix_sb[:, (off_cols + s0) * 8:
                                          (off_cols + s1) * 8],
                            num_idxs=(s1 - s0) * P,
                            num_idxs_reg=(s1 - s0) * P,
                            elem_size=EW, single_packet=False, queue_num=q)
                    EWF = EW // 2                      # f32 view width
                    AS0 = F // 2                       # a_s f32 col
                    ownf = own[:].bitcast(dt.float32)
                    ownv = ownf.rearrange("p (m e) -> p m e", e=EWF)
                    # self logits
                    eS = wp.tile([P, GSZ * NH], dt.float32, tag="eS")
                    nc.vector.tensor_tensor(
                        out=eS[:, :M * NH].rearrange("p (m h) -> p m h", h=NH),
                        in0=ownv[:, :M, AS0:AS0 + NH],
                        in1=ownv[:, :M, AS0 + NH:AS0 + 2 * NH],
                        op=mybir.AluOpType.add)
                    nc.vector.scalar_tensor_tensor(
                        out=eS[:, :M * NH], in0=eS[:, :M * NH],
                        scalar=NEG_SLOPE, in1=eS[:, :M * NH],
                        op0=mybir.AluOpType.mult, op1=mybir.AluOpType.max)
                    wS = wp.tile([P, GSZ * NH], dt.float32, tag="wS")
                    nc.scalar.activation(wS[:, :M * NH], eS[:, :M * NH],
                                         mybir.ActivationFunctionType.Exp)
                    # edge logits
                    et = wp.tile([P, MAXS * NH], dt.float32, tag="et")
                    payf = pay[:].bitcast(dt.float32)
                    a_s = payf.rearrange("p (s e) -> p s e", e=EWF)
                    for mi in range(M):
                        o0, o1 = int(goff[g][mi]), int(goff[g][mi + 1])
                        nc.vector.tensor_tensor(
                            out=et[:, o0 * NH:o1 * NH]
                                .rearrange("p (s h) -> p s h", h=NH),
                            in0=a_s[:, o0:o1, AS0:AS0 + NH],
                            in1=ownv[:, mi, AS0 + NH:AS0 + 2 * NH]
                                .unsqueeze(1).to_broadcast([P, o1 - o0, NH]),
                            op=mybir.AluOpType.add)
                    nc.vector.scalar_tensor_tensor(
                        out=et[:, :S * NH], in0=et[:, :S * NH],
                        scalar=NEG_SLOPE, in1=et[:, :S * NH],
                        op0=mybir.AluOpType.mult, op1=mybir.AluOpType.max)
                    wf = wp.tile([P, MAXS * NH], dt.float32, tag="wf")
                    nc.scalar.activation(wf[:, :S * NH], et[:, :S * NH],
                                         mybir.ActivationFunctionType.Exp)
                    # denominators + normalize weights in place
                    st = wp.tile([P, GSZ * NH], dt.float32, tag="st")
                    for mi in range(M):
                        o0, o1 = int(goff[g][mi]), int(goff[g][mi + 1])
                        nc.vector.tensor_reduce(
                            out=st[:, mi * NH:(mi + 1) * NH],
                            in_=wf[:, o0 * NH:o1 * NH]
                                .rearrange("p (s h) -> p h s", h=NH),
                            axis=mybir.AxisListType.X, op=mybir.AluOpType.add)
                    nc.vector.tensor_add(out=st[:, :M * NH], in0=st[:, :M * NH],
                                         in1=wS[:, :M * NH])
                    nc.vector.tensor_scalar_max(out=st[:, :M * NH],
                                                in0=st[:, :M * NH],
                                                scalar1=1e-30)
                    rc = wp.tile([P, GSZ * NH], dt.float32, tag="rc")
                    nc.vector.reciprocal(out=rc[:, :M * NH], in_=st[:, :M * NH])
                    for mi in range(M):
                        o0, o1 = int(goff[g][mi]), int(goff[g][mi + 1])
                        nc.vector.tensor_tensor(
                            out=wf[:, o0 * NH:o1 * NH]
                                .rearrange("p (s h) -> p s h", h=NH),
                            in0=wf[:, o0 * NH:o1 * NH]
                                .rearrange("p (s h) -> p s h", h=NH),
                            in1=rc[:, mi * NH:(mi + 1) * NH].unsqueeze(1)
                                .to_broadcast([P, o1 - o0, NH]),
                            op=mybir.AluOpType.mult)
                    nc.vector.tensor_tensor(
                        out=wS[:, :M * NH], in0=wS[:, :M * NH],
                        in1=rc[:, :M * NH], op=mybir.AluOpType.mult)
                    # cast alpha to bf16
                    wb = wp.tile([P, MAXS * NH], dt.bfloat16, tag="wb")
                    nc.vector.tensor_copy(out=wb[:, :S * NH], in_=wf[:, :S * NH])
                    wSb = wp.tile([P, GSZ * NH], dt.bfloat16, tag="wSb")
                    nc.vector.tensor_copy(out=wSb[:, :M * NH],
                                          in_=wS[:, :M * NH])
                    # self multiply in place ((c,h)-interleaved)
                    CPP = F // NH
                    oh = own[:].rearrange("p (m e) -> p m e", e=EW)[:, :M, :F] \
                        .rearrange("p m (c h) -> p m c h", h=NH)
                    nc.vector.tensor_tensor(
                        out=oh, in0=oh,
                        in1=wSb[:, :M * NH]
                            .rearrange("p (m h) -> p m h", h=NH)
                            .unsqueeze(2).to_broadcast([P, M, CPP, NH]),
                        op=mybir.AluOpType.mult)
                    # big multiply in place
                    hv = pay[:].rearrange("p (s e) -> p s e", e=EW)[:, :S, :F] \
                        .rearrange("p s (c h) -> p s c h", h=NH)
                    nc.vector.tensor_tensor(
                        out=hv, in0=hv,
                        in1=wb[:, :S * NH]
                            .rearrange("p (s h) -> p s h", h=NH)
                            .unsqueeze(2).to_broadcast([P, S, CPP, NH]),
                        op=mybir.AluOpType.mult)
                    # per member: 2-level tree + matmul finish
                    res_sb = hp.tile([P, GSZ * F], dt.bfloat16, tag="res")
                    for mi, t in enumerate(mem):
                        o0 = int(goff[g][mi])
                        d = int(d_t[t])
                        buf, base, stride, cnt = pay, o0, EW, d
                        for lvl in range(2):
                            if cnt < 2:
                                break
                            half, odd = cnt // 2, cnt % 2
                            dst = trp.tile([P, (T1 if lvl == 0 else T2) * F],
                                           dt.bfloat16, tag=f"tr{lvl}")
                            src = buf[:, base * stride:(base + cnt) * stride] \
                                .rearrange("p (s e) -> p s e", e=stride)
                            nc.vector.tensor_tensor(
                                out=dst[:, :half * F]
                                    .rearrange("p (s f) -> p s f", f=F),
                                in0=src[:, 0:2 * half:2, :F],
                                in1=src[:, 1:2 * half:2, :F],
                                op=mybir.AluOpType.add)
                            if odd:
                                nc.vector.tensor_copy(
                                    out=dst[:, half * F:(half + 1) * F],
                                    in_=buf[:, (base + cnt - 1) * stride:
                                            (base + cnt - 1) * stride + F])
                            buf, base, stride, cnt = dst, 0, F, half + odd
                        U = psU.tile([P, F], dt.float32, tag="U")
                        nc.tensor.matmul(U[:], lhsT=ide[:],
                                         rhs=own[:, mi * EW:mi * EW + F],
                                         start=True, stop=False)
                        for jj in range(cnt):
                            nc.tensor.matmul(
                                U[:], lhsT=ide[:],
                                rhs=buf[:, (base + jj) * stride:
                                        (base + jj) * stride + F],
                                start=False, stop=(jj == cnt - 1))
                        nc.scalar.activation(res_sb[:, mi * F:(mi + 1) * F],
                                             U[:],
                                             mybir.ActivationFunctionType.Copy)
                    if layer == 1:
                        # +b1, ELU (no -1: via neg1 const), dense2, r2 write
                        rv = res_sb[:, :M * F].rearrange("p (m f) -> p m f", f=F)
                        nc.vector.tensor_tensor(
                            out=rv, in0=rv,
                            in1=b1_sb[:].unsqueeze(1).to_broadcast([P, M, F]),
                            op=mybir.AluOpType.add)
                        tmin = hp.tile([P, GSZ * F], dt.bfloat16, tag="tmin")
                        nc.vector.scalar_tensor_tensor(
                            out=tmin[:, :M * F], in0=res_sb[:, :M * F],
                            scalar=0.0, in1=res_sb[:, :M * F],
                            op0=mybir.AluOpType.mult, op1=mybir.AluOpType.min)
                        texp = hp.tile([P, GSZ * F], dt.bfloat16, tag="texp")
                        nc.scalar.activation(texp[:, :M * F], tmin[:, :M * F],
                                             mybir.ActivationFunctionType.Exp)
                        nc.vector.scalar_tensor_tensor(
                            out=res_sb[:, :M * F], in0=res_sb[:, :M * F],
                            scalar=0.0, in1=texp[:, :M * F],
                            op0=mybir.AluOpType.max, op1=mybir.AluOpType.add)
                        nc.vector.tensor_tensor(
                            out=rv, in0=rv,
                            in1=n1_sb[:].unsqueeze(1).to_broadcast([P, M, F]),
                            op=mybir.AluOpType.add)
                        for mi, t in enumerate(mem):
                            tp1 = psB.tile([P, P], dt.bfloat16, tag="tp1")
                            nc.tensor.transpose(
                                tp1[:], res_sb[:, mi * F:mi * F + P], ide[:])
                            tp2 = psB.tile([64, P], dt.bfloat16, tag="tp2")
                            nc.tensor.transpose(
                                tp2[:], res_sb[:, mi * F + P:(mi + 1) * F],
                                ide[:])
                            hT1 = hp.tile([P, P], dt.bfloat16, tag="hT1")
                            nc.vector.tensor_copy(out=hT1[:], in_=tp1[:])
                            hT2 = hp.tile([64, P], dt.bfloat16, tag="hT2")
                            nc.vector.tensor_copy(out=hT2[:], in_=tp2[:])
                            o2 = psO.tile([P, 66], dt.float32, tag="o2")
                            nc.tensor.matmul(o2[:], lhsT=hT1[:], rhs=w2a_sb[:],
                                             start=True, stop=False)
                            nc.tensor.matmul(o2[:], lhsT=hT2[:], rhs=w2b_sb[:],
                                             start=False, stop=True)
                            r2 = hp.tile([P, 68], dt.bfloat16, tag="r2")
                            nc.scalar.activation(
                                r2[:, :OUT], o2[:, :OUT],
                                mybir.ActivationFunctionType.Copy)
                            nc.vector.tensor_copy(
                                out=r2[:, OUT:OUT + 4].bitcast(dt.float32),
                                in_=o2[:, OUT:OUT + 2])
                            nc.scalar.dma_start(
                                out=shard2[t * P:(t + 1) * P, :68], in_=r2[:])
                    else:
                        for mi, t in enumerate(mem):
                            ot = hp.tile([P, OUT], dt.float32, tag="ot")
                            nc.vector.tensor_copy(
                                out=ot[:], in_=res_sb[:, mi * F:(mi + 1) * F])
                            nc.scalar.dma_start(
                                out=out2[t * P:(t + 1) * P, :], in_=ot[:])

            # ---------------- L1 edge phase
            with tc.tile_pool(name="own1", bufs=3) as ownp, \
                 tc.tile_pool(name="pay1", bufs=pay1_bufs) as payp, \
                 tc.tile_pool(name="wp1", bufs=2) as wp, \
                 tc.tile_pool(name="trp1", bufs=2) as trp, \
                 tc.tile_pool(name="hp1", bufs=3) as hp, \
                 tc.tile_pool(name="psU1", bufs=3, space="PSUM") as psU, \
                 tc.tile_pool(name="psB1", bufs=1, space="PSUM") as psB, \
                 tc.tile_pool(name="psO1", bufs=2, space="PSUM") as psO:
                edge_phase(1, ag1[BASE:, :], shard1, E1, F1, payp, ownp, wp,
                           trp, hp, psU, psB, psO, HEADS)

            # ---------------- AllGather layer-2 table (two halves)
            nc.gpsimd.collective_compute(
                "AllGather", mybir.AluOpType.bypass, replica_groups=RG,
                ins=[shard2[0:HP, :].opt()],
                outs=[ag2[0:NCORE * HP, :].opt()])
            nc.gpsimd.collective_compute(
                "AllGather", mybir.AluOpType.bypass, replica_groups=RG,
                ins=[shard2[HP:CB, :].opt()],
                outs=[ag2[NCORE * HP:NTAB, :].opt()])

            # ---------------- L2 edge phase
            with tc.tile_pool(name="own2", bufs=3) as ownp2, \
                 tc.tile_pool(name="pay2", bufs=pay2_bufs) as payp2, \
                 tc.tile_pool(name="wp2", bufs=2) as wp2, \
                 tc.tile_pool(name="trp2", bufs=2) as trp2, \
                 tc.tile_pool(name="hp2", bufs=3) as hp2, \
                 tc.tile_pool(name="psU2", bufs=4, space="PSUM") as psU2:
                edge_phase(2, ag2[BASE:, :], shard2, E2, OUT, payp2, ownp2,
                           wp2, trp2, hp2, psU2, None, None, 1)
    nc.compile()
    return nc


# ---------------------------------------------------------------- kernel
def kernel(x, edge_index, W1, att_src1, att_dst1, b1, W2, att_src2, att_dst2,
           b2, _emulate=False, _timing=None):
    x = np.asarray(x, np.float32)
    edge_index = np.asarray(edge_index)
    W1 = np.asarray(W1, np.float32)
    att_src1 = np.asarray(att_src1, np.float32)
    att_dst1 = np.asarray(att_dst1, np.float32)
    b1 = np.asarray(b1, np.float32)
    W2 = np.asarray(W2, np.float32)
    att_src2 = np.asarray(att_src2, np.float32)
    att_dst2 = np.asarray(att_dst2, np.float32)
    b2 = np.asarray(b2, np.float32)

    if _emulate:
        return emulate(x, edge_index, W1, att_src1, att_dst1, b1,
                       W2, att_src2, att_dst2, b2)

    from concourse.bass_utils import run_bass_kernel_spmd
    import time as _time

    pre = preprocess(edge_index)
    hw = host_weights(x, W1, att_src1, att_dst1, b1, W2, att_src2, att_dst2,
                      b2, pre["nid"])
    nc = _build(pre)
    maps = [dict(xT=hw["xTs"][c], W1e=hw["W1e"], W2e1=hw["W2e1"],
                 W2e2=hw["W2e2"], b1i=hw["b1i"], neg1=hw["neg1"],
                 sent1=hw["sent1"], sent2=hw["sent2"], idxg=pre["idxg"][c])
            for c in range(NCORE)]

    trace = _timing is not None
    res = None
    for attempt in range(3):
        try:
            res = run_bass_kernel_spmd(nc, maps, core_ids=list(range(NCORE)),
                                       trace=trace and attempt == 0)
            break
        except Exception:
            if attempt == 2:
                raise
            _time.sleep(45)

    nid = pre["nid"]
    out = np.zeros((N, OUT), np.float32)
    for c in range(NCORE):
        o = res.results[c]["out2"]
        nn = nid[c].reshape(-1)
        valid = nn >= 0
        out[nn[valid]] = o[valid] + b2[None, :]

    if _timing is not None:
        _timing["neff1_ns"] = res.exec_time_ns
        _timing["neff2_ns"] = 0
    return out


# revision 7
# speedup vs baseline: 1.0331x; 1.0331x over previous
"""2-layer GAT (PyG-style) on TRN2, 8 NeuronCores — single-NEFF version.

Strategy: nodes sorted by in-degree (self-loops excluded) and dealt
round-robin across the 8 cores into 49 tiles/core of 128 nodes. Table rows
live in DRAM in (core, tile, partition) order, ROTATED per core so each
core's own tiles are block 0 (static offsets for self-loop rows and per-tile
a_d loads). Per-edge source rows are fetched with dma_gather in slot-major
order (dst node == partition), striped across the 4 SWDGE queues per
tile-group. Self-loop rows and per-tile a_d come by direct DMA (no gather).

Aggregation: DVE multiplies gathered rows by edge weights in place, then the
segment-sum runs on the TensorEngine as PSUM-accumulating matmuls with a
static identity lhsT (slot-major => dst == partition). Self-loop rows arrive
by direct DMA and join the same PSUM accumulation. f32 accumulation.

Layer-2 table (h2 | a_s2 | a_d2, 256B rows) is assembled on-device with two
AllGather collectives (first half overlaps the tail of layer-1 compute); own
rows are read from the local shard at static offsets.

Feature order inside rows is (c,h)-interleaved (pos i = c*3+h) so the big
per-edge multiply has every operand at innermost stride 1.
"""
import numpy as np
import ml_dtypes

N = 50000
E = 800000
IN = 128
HID = 64
HEADS = 3
OUT = 64
NCORE = 8
P = 128
T = 49                              # tiles per core
CB = T * P + P                      # 6400 rows per core block (incl pad tile)
NTAB = NCORE * CB                   # 51200
SENTROW = 51072                     # block-7 pad-tile row (same local & global)
BASE = 32768
E1 = 256                            # L1 row elems (bf16) = 512B
E2 = 128                            # L2 row elems (bf16) = 256B
F1 = HEADS * HID                    # 192
NEG_SLOPE = 0.2
NQ = 4                              # SWDGE queues
GSZ = 4                             # tiles per gather group
XC = 7                              # dense1 tiles per chunk (49 = 7*7)
HALF_T = 25                         # tiles 0..24 -> AG region A, 25..48 -> B

bf16 = ml_dtypes.bfloat16

# interleaved feature order: row position i=(c*3+h) holds feature f=h*64+c
_POS = np.arange(F1)
PERM = (_POS % HEADS) * HID + (_POS // HEADS)


def _pack_idx(rows_flat):
    """rows_flat int64[nidx] (local table rows, nidx%128==0) -> int16
    [128, nidx//16] wrap-16 layout replicated across the 8 Q7 groups."""
    v = (rows_flat - BASE).astype(np.int16)
    cid = len(v) // 16
    block = v.reshape(cid, 16).T
    return np.tile(block, (8, 1))


def _g2row(c, t, p):
    """Global AG-table row for node position (c,t,p): two half regions."""
    t = np.asarray(t)
    return np.where(t < HALF_T,
                    c * 3200 + t * 128 + p,
                    25600 + c * 3200 + (t - HALF_T) * 128 + p)


def preprocess(edge_index):
    src = edge_index[0].astype(np.int64)
    dst = edge_index[1].astype(np.int64)
    deg = np.bincount(dst, minlength=N)            # in-degree, no self loops
    order = np.argsort(-deg, kind="stable")
    eorder = np.argsort(dst, kind="stable")
    esrc = src[eorder]
    eptr = np.zeros(N + 1, np.int64)
    eptr[1:] = np.cumsum(deg)

    # node placement: lowest-degree nodes reserved for partition 127
    n127 = T * NCORE
    main = order[:N - n127]
    tail = order[N - n127:]
    SENT = -1
    nid = np.full((NCORE, T, P), SENT, np.int64)
    MP = P - 1
    for t in range(T):
        blk = main[t * NCORE * MP: (t + 1) * NCORE * MP]
        for c in range(NCORE):
            sl = blk[c::NCORE]
            nid[c, t, :len(sl)] = sl
        tb = tail[t * NCORE: (t + 1) * NCORE]
        for c in range(NCORE):
            if c < len(tb):
                nid[c, t, P - 1] = tb[c]

    # position maps
    posc = np.zeros(N, np.int64)
    post = np.zeros(N, np.int64)
    posp = np.zeros(N, np.int64)
    for c in range(NCORE):
        for t in range(T):
            nn = nid[c, t]
            v = nn != SENT
            posc[nn[v]] = c
            post[nn[v]] = t
            posp[nn[v]] = np.nonzero(v)[0]

    # per-tile max degree (p0 holds each core's max; p127 handled below)
    dmax = np.zeros(T, np.int64)
    d127 = np.zeros(T, np.int64)
    for t in range(T):
        for c in range(NCORE):
            if nid[c, t, 0] != SENT:
                dmax[t] = max(dmax[t], deg[nid[c, t, 0]])
            if nid[c, t, P - 1] != SENT:
                d127[t] = max(d127[t], deg[nid[c, t, P - 1]])
    d_t = np.maximum(dmax, 1)

    # groups: greedy balance by slot count into ceil(T/GSZ) bins
    ng = (T + GSZ - 1) // GSZ
    while True:
        bins = [[] for _ in range(ng)]
        load = np.zeros(ng, np.int64)
        cnt = np.zeros(ng, np.int64)
        for t in np.argsort(-d_t, kind="stable"):
            elig = np.nonzero(cnt < GSZ)[0]
            b = elig[np.argmin(load[elig])]
            bins[b].append(int(t))
            load[b] += d_t[t]
            cnt[b] += 1
        groups = [sorted(b) for b in bins]
        # stripe boundaries per group (4 queue-striped calls)
        stripes = []
        bnd_local = {t: set() for t in range(T)}
        for g, mem in enumerate(groups):
            S = int(sum(d_t[t] for t in mem))
            bs = sorted(set(max(1, round(S * q / NQ)) for q in range(1, NQ + 1)))
            st = []
            s0 = 0
            for b in bs:
                if b > s0:
                    st.append((s0, b))
                    s0 = b
            stripes.append(st)
            # map stripe-end slots to (tile, local slot)
            offs = np.cumsum([0] + [int(d_t[t]) for t in mem])
            for (a, b) in st:
                e = b - 1
                mi = int(np.searchsorted(offs, e, side="right")) - 1
                bnd_local[mem[mi]].add(int(e - offs[mi]))
        # feasibility: p127 edges must fit in non-boundary slots
        ok = True
        for t in range(T):
            avail = int(d_t[t]) - len(bnd_local[t])
            if d127[t] > avail:
                d_t[t] += d127[t] - avail
                ok = False
        if ok:
            break

    # per-group slot offsets (for SBUF layout / idx columns)
    goff = []
    for mem in groups:
        offs = np.cumsum([0] + [int(d_t[t]) for t in mem])
        goff.append(offs)
    CE = int(d_t.sum()) * 8            # idx columns

    # index grids, both layers
    idx1 = np.zeros((NCORE, P, CE), np.int16)
    idx2 = np.zeros((NCORE, P, CE), np.int16)
    for c in range(NCORE):
        b = (posc - c) % NCORE
        lrow = b * CB + post * 128 + posp           # L1 local rotated rows
        grow = _g2row(posc, post, posp)             # L2 global AG rows
        blocks1, blocks2 = [], []
        for g, mem in enumerate(groups):
            S = int(goff[g][-1])
            r1 = np.full((S, P), SENTROW, np.int64)
            r2 = np.full((S, P), SENTROW, np.int64)
            for mi, t in enumerate(mem):
                off = int(goff[g][mi])
                d = int(d_t[t])
                p127_slots = [s for s in range(d) if s not in bnd_local[t]]
                for p in range(P):
                    n = nid[c, t, p]
                    if n < 0:
                        continue
                    k = int(deg[n])
                    ee = esrc[eptr[n]:eptr[n] + k]
                    if p == P - 1:
                        sl = np.array(p127_slots[:k], np.int64)
                    else:
                        sl = np.arange(k)
                    r1[off + sl, p] = lrow[ee]
                    r2[off + sl, p] = grow[ee]
            blocks1.append(_pack_idx(r1.reshape(-1)))
            blocks2.append(_pack_idx(r2.reshape(-1)))
        idx1[c] = np.concatenate(blocks1, axis=1)
        idx2[c] = np.concatenate(blocks2, axis=1)

    return dict(order=order, nid=nid, d_t=d_t, deg=deg, esrc=esrc, eptr=eptr,
                groups=groups, stripes=stripes, goff=goff,
                idx1=idx1, idx2=idx2, posc=posc, post=post, posp=posp)


def host_weights(x, W1, att_src1, att_dst1, b1, W2, att_src2, att_dst2, b2,
                 nid):
    W1s = np.stack([W1[:, h * HID:(h + 1) * HID] @ att_src1[h]
                    for h in range(HEADS)], 1)       # [128,3]
    W1d = np.stack([W1[:, h * HID:(h + 1) * HID] @ att_dst1[h]
                    for h in range(HEADS)], 1)
    W1e = np.concatenate([W1[:, PERM], W1s, W1d], axis=1)        # [128,198]
    W2e = np.concatenate([W2, (W2 @ att_src2[0])[:, None],
                          (W2 @ att_dst2[0])[:, None]], 1)       # [192,66]
    W2e = W2e[PERM, :]
    # compact global X^T in (c,t,p) order
    Xg = np.zeros((IN, NCORE * T * P), np.float32)
    for c in range(NCORE):
        for t in range(T):
            nn = nid[c, t]
            v = nn >= 0
            colbase = (c * T + t) * P
            Xg[:, colbase + np.nonzero(v)[0]] = x[nn[v]].T
    Xg = Xg.astype(bf16)
    xTs = []
    for c in range(NCORE):
        xTs.append(np.concatenate(
            [Xg[:, ((c + b) % NCORE) * T * P:(((c + b) % NCORE) + 1) * T * P]
             for b in range(NCORE)], axis=1))
    sent1 = np.zeros(E1, bf16)
    sent1.view(np.float32)[96:99] = -1e30
    sent2 = np.zeros(E2, bf16)
    sent2.view(np.float32)[32] = -1e30
    b1i = b1[PERM].astype(bf16)
    return dict(xTs=xTs, W1e=W1e.astype(bf16),
                W2e1=W2e[:128].astype(bf16), W2e2=W2e[128:].astype(bf16),
                sent1=sent1.reshape(1, E1), sent2=sent2.reshape(1, E2),
                b1i=np.tile(b1i, (P, 1)),
                b2b=np.tile(b2.astype(np.float32), (P, 1)))


# ---------------------------------------------------------------- emulation
def _bf(a):
    return a.astype(bf16).astype(np.float32)


def emulate(x, edge_index, W1, att_src1, att_dst1, b1, W2, att_src2, att_dst2,
            b2):
    pre = preprocess(edge_index)
    hw = host_weights(x, W1, att_src1, att_dst1, b1, W2, att_src2, att_dst2,
                      b2, pre["nid"])
    nid, d_t = pre["nid"], pre["d_t"]
    deg, esrc, eptr = pre["deg"], pre["esrc"], pre["eptr"]
    groups = pre["groups"]

    # dense1 (bf16 in, f32 psum)
    W1ef = hw["W1e"].astype(np.float32)
    b1f = hw["b1i"].astype(np.float32)[0]
    W2e1f = hw["W2e1"].astype(np.float32)
    W2e2f = hw["W2e2"].astype(np.float32)
    b2f = hw["b2b"][0]

    # table in GLOBAL node order (rotation only changes addressing)
    tab_h = np.zeros((N, F1), np.float32)
    tab_as = np.zeros((N, HEADS), np.float32)
    tab_ad = np.zeros((N, HEADS), np.float32)
    for c in range(NCORE):
        for t in range(T):
            nn = nid[c, t]
            v = nn >= 0
            xx = _bf(x[nn[v]])
            H = xx @ W1ef                      # f32 accum of bf16 inputs
            tab_h[nn[v]] = _bf(H[:, :F1])
            tab_as[nn[v]] = H[:, F1:F1 + 3]
            tab_ad[nn[v]] = H[:, F1 + 3:F1 + 6]

    tab2_h = np.zeros((N, OUT), np.float32)
    tab2_as = np.zeros((N, 1), np.float32)
    tab2_ad = np.zeros((N, 1), np.float32)

    def gat_layer(th, tas, tad, nf, layer):
        """th[N,nf] (perm'd for L1), tas/tad [N,H'] -> per-(c,t) outputs."""
        H_ = tas.shape[1]
        rep = nf // H_
        outs = np.zeros((NCORE, T, P, nf), np.float32)
        for c in range(NCORE):
            for t in range(T):
                d = int(d_t[t])
                rows = np.full((P, d), -1, np.int64)
                # boundary slots for p127
                g = next(gi for gi, mem in enumerate(groups) if t in mem)
                bset = set()
                offs = pre["goff"][g]
                for (a, bb) in pre["stripes"][g]:
                    e = bb - 1
                    mj = int(np.searchsorted(offs, e, side="right")) - 1
                    if groups[g][mj] == t:
                        bset.add(int(e - offs[mj]))
                p127_slots = [s for s in range(d) if s not in bset]
                for p in range(P):
                    n = nid[c, t, p]
                    if n < 0:
                        continue
                    k = int(deg[n])
                    sl = p127_slots[:k] if p == P - 1 else list(range(k))
                    rows[p, sl] = esrc[eptr[n]:eptr[n] + k]
                pad = rows < 0
                rr = np.where(pad, 0, rows)
                g_h = th[rr]                        # [P,d,nf] bf16-valued
                g_as = np.where(pad[:, :, None], -1e30, tas[rr])
                own = nid[c, t]
                ov = own >= 0
                oh = np.where(ov[:, None], th[np.where(ov, own, 0)], 0.0)
                oas = np.where(ov[:, None], tas[np.where(ov, own, 0)], -1e30)
                oad = np.where(ov[:, None], tad[np.where(ov, own, 0)], 0.0)
                # logits
                eE = g_as + oad[:, None, :]
                eE = np.maximum(eE, NEG_SLOPE * eE)
                wE = np.exp(eE)
                eS = oas + oad
                eS = np.maximum(eS, NEG_SLOPE * eS)
                wS = np.exp(eS)
                s = wE.sum(axis=1) + wS             # [P,H'] f32
                wEb = _bf(wE)
                wSb = _bf(wS)
                if layer == 1:
                    # interleaved: feature i=(cc*3+h) scaled by w[...,h]
                    wexp = np.repeat(wEb[:, :, None, :], HID, 2).reshape(P, d, nf)
                    sexp = np.repeat(wSb[:, None, :], HID, 1).reshape(P, nf)
                else:
                    wexp = np.repeat(wEb, rep, axis=2)
                    sexp = np.repeat(wSb, rep, axis=1)
                prod = _bf(g_h * wexp)
                prod[pad] = 0.0                     # pad rows gather sentinel
                sprod = _bf(oh * sexp)
                U = sprod.astype(np.float32) + prod.sum(axis=1, dtype=np.float32)
                r = _bf(1.0 / np.maximum(s, 1e-30))
                if layer == 1:
                    rexp = np.repeat(r[:, None, :], HID, 1).reshape(P, nf)
                    h1 = _bf(U * _bf(rexp))
                    h1 = _bf(h1 + b1f)
                    h1 = _bf(np.maximum(h1, 0) +
                             _bf(np.exp(np.minimum(h1, 0))) - 1)
                    outs[c, t] = h1
                else:
                    outs[c, t] = U * r + b2f
        return outs

    h1 = gat_layer(tab_h, tab_as, tab_ad, F1, 1)
    # dense2 (per tile, f32 accum of bf16)
    for c in range(NCORE):
        for t in range(T):
            o2 = _bf(h1[c, t]) @ np.concatenate([W2e1f, W2e2f], 0)
            nn = nid[c, t]
            v = nn >= 0
            tab2_h[nn[v]] = _bf(o2[v, :OUT])
            tab2_as[nn[v], 0] = o2[v, OUT]
            tab2_ad[nn[v], 0] = o2[v, OUT + 1]
    out = gat_layer(tab2_h, tab2_as, tab2_ad, OUT, 2)
    res = np.zeros((N, OUT), np.float32)
    for c in range(NCORE):
        for t in range(T):
            nn = nid[c, t]
            v = nn >= 0
            res[nn[v]] = out[c, t][v]
    return res


# ---------------------------------------------------------------- bass build
def _build(pre, pay1_bufs=3, pay2_bufs=4):
    import concourse.bacc as bacc
    import concourse.mybir as mybir
    import concourse.tile as tile
    from concourse.masks import make_identity

    d_t = pre["d_t"]
    groups, stripes, goff = pre["groups"], pre["stripes"], pre["goff"]
    CE = int(d_t.sum()) * 8
    MAXS = max(int(goff[g][-1]) for g in range(len(groups)))

    dt = mybir.dt
    nc = bacc.Bacc(num_devices=NCORE, num_swdge_queues=NQ)
    xT = nc.dram_tensor("xT", [IN, NCORE * T * P], dt.bfloat16, kind="ExternalInput")
    W1e = nc.dram_tensor("W1e", [IN, 198], dt.bfloat16, kind="ExternalInput")
    W2e1 = nc.dram_tensor("W2e1", [128, 66], dt.bfloat16, kind="ExternalInput")
    W2e2 = nc.dram_tensor("W2e2", [64, 66], dt.bfloat16, kind="ExternalInput")
    b1i = nc.dram_tensor("b1i", [P, F1], dt.bfloat16, kind="ExternalInput")
    b2b = nc.dram_tensor("b2b", [P, OUT], dt.float32, kind="ExternalInput")
    sent1 = nc.dram_tensor("sent1", [1, E1], dt.bfloat16, kind="ExternalInput")
    sent2 = nc.dram_tensor("sent2", [1, E2], dt.bfloat16, kind="ExternalInput")
    idx1 = nc.dram_tensor("idx1", [P, CE], dt.int16, kind="ExternalInput")
    idx2 = nc.dram_tensor("idx2", [P, CE], dt.int16, kind="ExternalInput")
    out2 = nc.dram_tensor("out2", [T * P, OUT], dt.float32, kind="ExternalOutput")
    tab1 = nc.dram_tensor("tab1", [NTAB, E1], dt.bfloat16)
    shard = nc.dram_tensor("shard", [CB, E2], dt.bfloat16)
    ag = nc.dram_tensor("ag", [NTAB, E2], dt.bfloat16, addr_space="Shared")

    with tile.TileContext(nc) as tc:
        with tc.tile_pool(name="const", bufs=1) as cp:
            w1_sb = cp.tile([IN, 198], dt.bfloat16)
            nc.sync.dma_start(out=w1_sb[:], in_=W1e[:, :])
            w2a_sb = cp.tile([128, 66], dt.bfloat16)
            nc.sync.dma_start(out=w2a_sb[:], in_=W2e1[:, :])
            w2b_sb = cp.tile([64, 66], dt.bfloat16)
            nc.sync.dma_start(out=w2b_sb[:], in_=W2e2[:, :])
            b1_sb = cp.tile([P, F1], dt.bfloat16)
            nc.sync.dma_start(out=b1_sb[:], in_=b1i[:, :])
            b2_sb = cp.tile([P, OUT], dt.float32)
            nc.sync.dma_start(out=b2_sb[:], in_=b2b[:, :])
            ide = cp.tile([P, P], dt.bfloat16)
            make_identity(nc, ide[:])
            i1_sb = cp.tile([P, CE], dt.int16)
            nc.sync.dma_start(out=i1_sb[:], in_=idx1[:, :])
            i2_sb = cp.tile([P, CE], dt.int16)
            nc.sync.dma_start(out=i2_sb[:], in_=idx2[:, :])
            sent1_sb = cp.tile([1, E1], dt.bfloat16)
            nc.sync.dma_start(out=sent1_sb[:], in_=sent1[:, :])
            nc.sync.dma_start(out=tab1[SENTROW:SENTROW + 1, :], in_=sent1_sb[:])
            sent2_sb = cp.tile([1, E2], dt.bfloat16)
            nc.sync.dma_start(out=sent2_sb[:], in_=sent2[:, :])
            nc.sync.dma_start(out=shard[T * P:T * P + 1, :], in_=sent2_sb[:])

            # ---------------- dense1: all 50176 rows, rotated layout
            with tc.tile_pool(name="xp", bufs=3) as xp, \
                 tc.tile_pool(name="rowp", bufs=3) as rowp, \
                 tc.tile_pool(name="psD", bufs=4, space="PSUM") as psD:
                for ch in range(NCORE * XC):
                    blk, j = divmod(ch, XC)
                    xch = xp.tile([P, XC * P], dt.bfloat16, tag="x")
                    nc.sync.dma_start(
                        out=xch[:], in_=xT[:, ch * XC * P:(ch + 1) * XC * P])
                    rt = rowp.tile([P, XC * E1], dt.bfloat16, tag="rt")
                    for k in range(XC):
                        pt = psD.tile([P, 198], dt.float32, tag="d1")
                        nc.tensor.matmul(pt[:], lhsT=xch[:, k * P:(k + 1) * P],
                                         rhs=w1_sb[:], start=True, stop=True)
                        nc.scalar.activation(rt[:, k * E1:k * E1 + F1],
                                             pt[:, :F1],
                                             mybir.ActivationFunctionType.Copy)
                        nc.vector.tensor_copy(
                            out=rt[:, k * E1 + F1:k * E1 + F1 + 12]
                                .bitcast(dt.float32),
                            in_=pt[:, F1:198])
                    dst = tab1[blk * CB + j * XC * P:
                               blk * CB + (j + 1) * XC * P, :]
                    nc.sync.dma_start(
                        out=dst.rearrange("(k p) e -> p k e", p=P),
                        in_=rt[:].rearrange("p (k e) -> p k e", e=E1))

            # ---------------- L1 edge phase
            tab_lo = tab1[BASE:, :]
            with tc.tile_pool(name="own", bufs=3) as ownp, \
                 tc.tile_pool(name="pay", bufs=pay1_bufs) as payp, \
                 tc.tile_pool(name="wp", bufs=3) as wp, \
                 tc.tile_pool(name="hp", bufs=3) as hp, \
                 tc.tile_pool(name="psU", bufs=3, space="PSUM") as psU, \
                 tc.tile_pool(name="psB", bufs=1, space="PSUM") as psB, \
                 tc.tile_pool(name="psO", bufs=2, space="PSUM") as psO:
                for g, mem in enumerate(groups):
                    M = len(mem)
                    S = int(goff[g][-1])
                    off_cols = int(np.sum([goff[gg][-1] for gg in range(g)]))
                    own = ownp.tile([P, GSZ * E1], dt.bfloat16, tag="own")
                    for mi, t in enumerate(mem):
                        nc.sync.dma_start(
                            out=own[:, mi * E1:(mi + 1) * E1],
                            in_=tab1[t * P:(t + 1) * P, :])
                    pay = payp.tile([P, MAXS * E1], dt.bfloat16, tag="pay")
                    for q, (s0, s1) in enumerate(stripes[g]):
                        nc.gpsimd.dma_gather(
                            out_ap=pay[:, s0 * E1:s1 * E1]
                                .rearrange("p (s e) -> p s e", e=E1),
                            in_ap=tab_lo,
                            idxs_ap=i1_sb[:, (off_cols + s0) * 8:
                                          (off_cols + s1) * 8],
                            num_idxs=(s1 - s0) * P,
                            num_idxs_reg=(s1 - s0) * P,
                            elem_size=E1, single_packet=False, queue_num=q)
                    ownf = own[:].bitcast(dt.float32)
                    ownv = ownf.rearrange("p (m e) -> p m e", e=128)
                    # self logits
                    eS = wp.tile([P, GSZ * 3], dt.float32, tag="eS")
                    nc.vector.tensor_tensor(
                        out=eS[:, :M * 3].rearrange("p (m h) -> p m h", h=3),
                        in0=ownv[:, :M, 96:99], in1=ownv[:, :M, 99:102],
                        op=mybir.AluOpType.add)
                    eS2 = wp.tile([P, GSZ * 3], dt.float32, tag="eS2")
                    nc.vector.scalar_tensor_tensor(
                        out=eS2[:, :M * 3], in0=eS[:, :M * 3], scalar=NEG_SLOPE,
                        in1=eS[:, :M * 3], op0=mybir.AluOpType.mult,
                        op1=mybir.AluOpType.max)
                    wS = wp.tile([P, GSZ * 3], dt.float32, tag="wS")
                    nc.scalar.activation(wS[:, :M * 3], eS2[:, :M * 3],
                                         mybir.ActivationFunctionType.Exp)
                    wSb = wp.tile([P, GSZ * 3], dt.bfloat16, tag="wSb")
                    nc.vector.tensor_copy(out=wSb[:, :M * 3], in_=wS[:, :M * 3])
                    # self multiply in place (interleaved (c,h))
                    oh = own[:].rearrange("p (m e) -> p m e", e=E1)[:, :M, :F1] \
                        .rearrange("p m (c h) -> p m c h", h=3)
                    nc.vector.tensor_tensor(
                        out=oh,
                        in0=oh,
                        in1=wSb[:, :M * 3].rearrange("p (m h) -> p m h", h=3)
                            .unsqueeze(2).to_broadcast([P, M, HID, 3]),
                        op=mybir.AluOpType.mult)
                    # edge logits (per-member add, group-wide rest)
                    et = wp.tile([P, MAXS * 3], dt.float32, tag="et")
                    payf = pay[:].bitcast(dt.float32)
                    a_s = payf.rearrange("p (s e) -> p s e", e=128)
                    for mi, t in enumerate(mem):
                        o0, o1 = int(goff[g][mi]), int(goff[g][mi + 1])
                        nc.vector.tensor_tensor(
                            out=et[:, o0 * 3:o1 * 3]
                                .rearrange("p (s h) -> p s h", h=3),
                            in0=a_s[:, o0:o1, 96:99],
                            in1=ownv[:, mi, 99:102].unsqueeze(1)
                                .to_broadcast([P, o1 - o0, 3]),
                            op=mybir.AluOpType.add)
                    et2 = wp.tile([P, MAXS * 3], dt.float32, tag="et2")
                    nc.vector.scalar_tensor_tensor(
                        out=et2[:, :S * 3], in0=et[:, :S * 3], scalar=NEG_SLOPE,
                        in1=et[:, :S * 3], op0=mybir.AluOpType.mult,
                        op1=mybir.AluOpType.max)
                    wf = wp.tile([P, MAXS * 3], dt.float32, tag="wf")
                    nc.scalar.activation(wf[:, :S * 3], et2[:, :S * 3],
                                         mybir.ActivationFunctionType.Exp)
                    wb = wp.tile([P, MAXS * 3], dt.bfloat16, tag="wb")
                    nc.vector.tensor_copy(out=wb[:, :S * 3], in_=wf[:, :S * 3])
                    # big multiply in place
                    h_view = pay[:].rearrange("p (s e) -> p s e", e=E1)[:, :S, :F1] \
                        .rearrange("p s (c h) -> p s c h", h=3)
                    w_view = wb[:, :S * 3].rearrange("p (s h) -> p s h", h=3) \
                        .unsqueeze(2).to_broadcast([P, S, HID, 3])
                    nc.vector.tensor_tensor(out=h_view, in0=h_view, in1=w_view,
                                            op=mybir.AluOpType.mult)
                    # denominators (per member) + self
                    st = wp.tile([P, GSZ * 3], dt.float32, tag="st")
                    for mi, t in enumerate(mem):
                        o0, o1 = int(goff[g][mi]), int(goff[g][mi + 1])
                        nc.vector.tensor_reduce(
                            out=st[:, mi * 3:(mi + 1) * 3],
                            in_=wf[:, o0 * 3:o1 * 3]
                                .rearrange("p (s h) -> p h s", h=3),
                            axis=mybir.AxisListType.X, op=mybir.AluOpType.add)
                    nc.vector.tensor_add(out=st[:, :M * 3], in0=st[:, :M * 3],
                                         in1=wS[:, :M * 3])
                    nc.vector.tensor_scalar_max(out=st[:, :M * 3],
                                                in0=st[:, :M * 3], scalar1=1e-30)
                    rc = wp.tile([P, GSZ * 3], dt.float32, tag="rc")
                    nc.vector.reciprocal(out=rc[:, :M * 3], in_=st[:, :M * 3])
                    rcb = wp.tile([P, GSZ * 3], dt.bfloat16, tag="rcb")
                    nc.vector.tensor_copy(out=rcb[:, :M * 3], in_=rc[:, :M * 3])
                    # accumulate + psum copy per member
                    h1g = hp.tile([P, GSZ * F1], dt.bfloat16, tag="h1g")
                    for mi, t in enumerate(mem):
                        o0 = int(goff[g][mi])
                        d = int(d_t[t])
                        U = psU.tile([P, F1], dt.float32, tag="U")
                        nc.tensor.matmul(U[:], lhsT=ide[:],
                                         rhs=own[:, mi * E1:mi * E1 + F1],
                                         start=True, stop=False)
                        for s in range(d):
                            nc.tensor.matmul(
                                U[:], lhsT=ide[:],
                                rhs=pay[:, (o0 + s) * E1:(o0 + s) * E1 + F1],
                                start=False, stop=(s == d - 1))
                        nc.scalar.activation(h1g[:, mi * F1:(mi + 1) * F1],
                                             U[:],
                                             mybir.ActivationFunctionType.Copy)
                    # group-wide normalize + bias + ELU
                    h1v = h1g[:, :M * F1].rearrange("p (m c h) -> p m c h", h=3, c=HID)
                    nc.vector.tensor_tensor(
                        out=h1v, in0=h1v,
                        in1=rcb[:, :M * 3].rearrange("p (m h) -> p m h", h=3)
                            .unsqueeze(2).to_broadcast([P, M, HID, 3]),
                        op=mybir.AluOpType.mult)
                    nc.vector.tensor_tensor(
                        out=h1g[:, :M * F1].rearrange("p (m f) -> p m f", f=F1),
                        in0=h1g[:, :M * F1].rearrange("p (m f) -> p m f", f=F1),
                        in1=b1_sb[:].unsqueeze(1).to_broadcast([P, M, F1]),
                        op=mybir.AluOpType.add)
                    tmin = hp.tile([P, GSZ * F1], dt.bfloat16, tag="tmin")
                    nc.vector.tensor_scalar_min(out=tmin[:, :M * F1],
                                                in0=h1g[:, :M * F1], scalar1=0.0)
                    texp = hp.tile([P, GSZ * F1], dt.bfloat16, tag="texp")
                    nc.scalar.activation(texp[:, :M * F1], tmin[:, :M * F1],
                                         mybir.ActivationFunctionType.Exp)
                    nc.vector.scalar_tensor_tensor(
                        out=h1g[:, :M * F1], in0=h1g[:, :M * F1], scalar=0.0,
                        in1=texp[:, :M * F1], op0=mybir.AluOpType.max,
                        op1=mybir.AluOpType.add)
                    nc.vector.tensor_scalar_add(out=h1g[:, :M * F1],
                                                in0=h1g[:, :M * F1],
                                                scalar1=-1.0)
                    # dense2 per member
                    for mi, t in enumerate(mem):
                        tp1 = psB.tile([P, P], dt.bfloat16, tag="tp1")
                        nc.tensor.transpose(tp1[:], h1g[:, mi * F1:mi * F1 + P],
                                            ide[:])
                        tp2 = psB.tile([64, P], dt.bfloat16, tag="tp2")
                        nc.tensor.transpose(tp2[:],
                                            h1g[:, mi * F1 + P:(mi + 1) * F1],
                                            ide[:])
                        hT1 = hp.tile([P, P], dt.bfloat16, tag="hT1")
                        nc.vector.tensor_copy(out=hT1[:], in_=tp1[:])
                        hT2 = hp.tile([64, P], dt.bfloat16, tag="hT2")
                        nc.vector.tensor_copy(out=hT2[:], in_=tp2[:])
                        o2 = psO.tile([P, 66], dt.float32, tag="o2")
                        nc.tensor.matmul(o2[:], lhsT=hT1[:], rhs=w2a_sb[:],
                                         start=True, stop=False)
                        nc.tensor.matmul(o2[:], lhsT=hT2[:], rhs=w2b_sb[:],
                                         start=False, stop=True)
                        r2 = hp.tile([P, 68], dt.bfloat16, tag="r2")
                        nc.scalar.activation(r2[:, :OUT], o2[:, :OUT],
                                             mybir.ActivationFunctionType.Copy)
                        nc.vector.tensor_copy(
                            out=r2[:, OUT:OUT + 4].bitcast(dt.float32),
                            in_=o2[:, OUT:OUT + 2])
                        nc.sync.dma_start(out=shard[t * P:(t + 1) * P, :68],
                                          in_=r2[:])

            # ---------------- AllGather table2 (two halves)
            nc.gpsimd.collective_compute(
                "AllGather", mybir.AluOpType.bypass,
                replica_groups=[list(range(NCORE))],
                ins=[shard[0:HALF_T * P, :].opt()],
                outs=[ag[0:NCORE * HALF_T * P, :].opt()])
            nc.gpsimd.collective_compute(
                "AllGather", mybir.AluOpType.bypass,
                replica_groups=[list(range(NCORE))],
                ins=[shard[HALF_T * P:CB, :].opt()],
                outs=[ag[NCORE * HALF_T * P:NTAB, :].opt()])

            # ---------------- L2 edge phase
            ag_lo = ag[BASE:, :]
            with tc.tile_pool(name="own2", bufs=3) as ownp2, \
                 tc.tile_pool(name="pay2", bufs=pay2_bufs) as payp2, \
                 tc.tile_pool(name="wp2", bufs=3) as wp2, \
                 tc.tile_pool(name="op2", bufs=3) as op2, \
                 tc.tile_pool(name="psU2", bufs=4, space="PSUM") as psU2:
                for g, mem in enumerate(groups):
                    M = len(mem)
                    S = int(goff[g][-1])
                    off_cols = int(np.sum([goff[gg][-1] for gg in range(g)]))
                    own = ownp2.tile([P, GSZ * E2], dt.bfloat16, tag="own")
                    for mi, t in enumerate(mem):
                        nc.sync.dma_start(
                            out=own[:, mi * E2:(mi + 1) * E2],
                            in_=shard[t * P:(t + 1) * P, :])
                    pay = payp2.tile([P, MAXS * E2], dt.bfloat16, tag="pay")
                    for q, (s0, s1) in enumerate(stripes[g]):
                        nc.gpsimd.dma_gather(
                            out_ap=pay[:, s0 * E2:s1 * E2]
                                .rearrange("p (s e) -> p s e", e=E2),
                            in_ap=ag_lo,
                            idxs_ap=i2_sb[:, (off_cols + s0) * 8:
                                          (off_cols + s1) * 8],
                            num_idxs=(s1 - s0) * P,
                            num_idxs_reg=(s1 - s0) * P,
                            elem_size=E2, single_packet=False, queue_num=q)
                    ownf = own[:].bitcast(dt.float32)
                    ownv = ownf.rearrange("p (m e) -> p m e", e=64)
                    eS = wp2.tile([P, GSZ], dt.float32, tag="eS")
                    nc.vector.tensor_tensor(
                        out=eS[:, :M].unsqueeze(2),
                        in0=ownv[:, :M, 32:33], in1=ownv[:, :M, 33:34],
                        op=mybir.AluOpType.add)
                    eS2 = wp2.tile([P, GSZ], dt.float32, tag="eS2")
                    nc.vector.scalar_tensor_tensor(
                        out=eS2[:, :M], in0=eS[:, :M], scalar=NEG_SLOPE,
                        in1=eS[:, :M], op0=mybir.AluOpType.mult,
                        op1=mybir.AluOpType.max)
                    wS = wp2.tile([P, GSZ], dt.float32, tag="wS")
                    nc.scalar.activation(wS[:, :M], eS2[:, :M],
                                         mybir.ActivationFunctionType.Exp)
                    wSb = wp2.tile([P, GSZ], dt.bfloat16, tag="wSb")
                    nc.vector.tensor_copy(out=wSb[:, :M], in_=wS[:, :M])
                    oh = own[:].rearrange("p (m e) -> p m e", e=E2)[:, :M, :OUT]
                    nc.vector.tensor_tensor(
                        out=oh, in0=oh,
                        in1=wSb[:, :M].unsqueeze(2).to_broadcast([P, M, OUT]),
                        op=mybir.AluOpType.mult)
                    et = wp2.tile([P, MAXS], dt.float32, tag="et")
                    payf = pay[:].bitcast(dt.float32)
                    a_s = payf.rearrange("p (s e) -> p s e", e=64)
                    for mi, t in enumerate(mem):
                        o0, o1 = int(goff[g][mi]), int(goff[g][mi + 1])
                        nc.vector.tensor_tensor(
                            out=et[:, o0:o1].unsqueeze(2),
                            in0=a_s[:, o0:o1, 32:33],
                            in1=ownv[:, mi, 33:34].unsqueeze(1)
                                .to_broadcast([P, o1 - o0, 1]),
                            op=mybir.AluOpType.add)
                    et2 = wp2.tile([P, MAXS], dt.float32, tag="et2")
                    nc.vector.scalar_tensor_tensor(
                        out=et2[:, :S], in0=et[:, :S], scalar=NEG_SLOPE,
                        in1=et[:, :S], op0=mybir.AluOpType.mult,
                        op1=mybir.AluOpType.max)
                    wf = wp2.tile([P, MAXS], dt.float32, tag="wf")
                    nc.scalar.activation(wf[:, :S], et2[:, :S],
                                         mybir.ActivationFunctionType.Exp)
                    wb = wp2.tile([P, MAXS], dt.bfloat16, tag="wb")
                    nc.vector.tensor_copy(out=wb[:, :S], in_=wf[:, :S])
                    h_view = pay[:].rearrange("p (s e) -> p s e", e=E2)[:, :S, :OUT]
                    w_view = wb[:, :S].unsqueeze(2).to_broadcast([P, S, OUT])
                    nc.vector.tensor_tensor(out=h_view, in0=h_view, in1=w_view,
                                            op=mybir.AluOpType.mult)
                    st = wp2.tile([P, GSZ], dt.float32, tag="st")
                    for mi, t in enumerate(mem):
                        o0, o1 = int(goff[g][mi]), int(goff[g][mi + 1])
                        nc.vector.tensor_reduce(
                            out=st[:, mi:mi + 1], in_=wf[:, o0:o1],
                            axis=mybir.AxisListType.X, op=mybir.AluOpType.add)
                    nc.vector.tensor_add(out=st[:, :M], in0=st[:, :M],
                                         in1=wS[:, :M])
                    nc.vector.tensor_scalar_max(out=st[:, :M], in0=st[:, :M],
                                                scalar1=1e-30)
                    rc = wp2.tile([P, GSZ], dt.float32, tag="rc")
                    nc.vector.reciprocal(out=rc[:, :M], in_=st[:, :M])
                    for mi, t in enumerate(mem):
                        o0 = int(goff[g][mi])
                        d = int(d_t[t])
                        U = psU2.tile([P, OUT], dt.float32, tag="U2")
                        nc.tensor.matmul(U[:], lhsT=ide[:],
                                         rhs=own[:, mi * E2:mi * E2 + OUT],
                                         start=True, stop=False)
                        for s in range(d):
                            nc.tensor.matmul(
                                U[:], lhsT=ide[:],
                                rhs=pay[:, (o0 + s) * E2:(o0 + s) * E2 + OUT],
                                start=False, stop=(s == d - 1))
                        ot = op2.tile([P, OUT], dt.float32, tag="ot")
                        nc.scalar.activation(ot[:], U[:],
                                             mybir.ActivationFunctionType.Copy)
                        nc.vector.tensor_scalar(
                            out=ot[:], in0=ot[:], scalar1=rc[:, mi:mi + 1],
                            scalar2=None, op0=mybir.AluOpType.mult)
                        nc.vector.tensor_add(out=ot[:], in0=ot[:], in1=b2_sb[:])
                        nc.sync.dma_start(out=out2[t * P:(t + 1) * P, :],
                                          in_=ot[:])
    nc.compile()
    return nc


# ---------------------------------------------------------------- kernel
def kernel(x, edge_index, W1, att_src1, att_dst1, b1, W2, att_src2, att_dst2,
           b2, _emulate=False, _timing=None):
    x = np.asarray(x, np.float32)
    edge_index = np.asarray(edge_index)
    W1 = np.asarray(W1, np.float32)
    att_src1 = np.asarray(att_src1, np.float32)
    att_dst1 = np.asarray(att_dst1, np.float32)
    b1 = np.asarray(b1, np.float32)
    W2 = np.asarray(W2, np.float32)
    att_src2 = np.asarray(att_src2, np.float32)
    att_dst2 = np.asarray(att_dst2, np.float32)
    b2 = np.asarray(b2, np.float32)

    if _emulate:
        return emulate(x, edge_index, W1, att_src1, att_dst1, b1,
                       W2, att_src2, att_dst2, b2)

    from concourse.bass_utils import run_bass_kernel_spmd
    import time as _time

    pre = preprocess(edge_index)
    hw = host_weights(x, W1, att_src1, att_dst1, b1, W2, att_src2, att_dst2,
                      b2, pre["nid"])
    nc = _build(pre)
    maps = [dict(xT=hw["xTs"][c], W1e=hw["W1e"], W2e1=hw["W2e1"],
                 W2e2=hw["W2e2"], b1i=hw["b1i"], b2b=hw["b2b"],
                 sent1=hw["sent1"], sent2=hw["sent2"],
                 idx1=pre["idx1"][c], idx2=pre["idx2"][c])
            for c in range(NCORE)]

    trace = _timing is not None
    res = None
    for attempt in range(3):
        try:
            res = run_bass_kernel_spmd(nc, maps, core_ids=list(range(NCORE)),
                                       trace=trace and attempt == 0)
            break
        except Exception:
            if attempt == 2:
                raise
            _time.sleep(45)

    nid = pre["nid"]
    out = np.zeros((N, OUT), np.float32)
    for c in range(NCORE):
        o = res.results[c]["out2"]
        nn = nid[c].reshape(-1)
        valid = nn >= 0
        out[nn[valid]] = o[valid]

    if _timing is not None:
        _timing["neff1_ns"] = res.exec_time_ns
        _timing["neff2_ns"] = 0
    return out


# revision 9
# speedup vs baseline: 1.0654x; 1.0312x over previous
"""2-layer GAT (PyG-style) on TRN2, 8 NeuronCores — single-NEFF version.

Strategy: nodes sorted by in-degree (self-loops excluded) and dealt
round-robin across the 8 cores into 49 tiles/core of 128 nodes. Table rows
live in DRAM in (core, tile, partition) order, ROTATED per core so each
core's own tiles are block 0 (static offsets for self-loop rows and per-tile
a_d loads). Per-edge source rows are fetched with dma_gather in slot-major
order (dst node == partition), striped across the 4 SWDGE queues per
tile-group. Self-loop rows and per-tile a_d come by direct DMA (no gather).

Aggregation: DVE multiplies gathered rows by edge weights in place, then the
segment-sum runs on the TensorEngine as PSUM-accumulating matmuls with a
static identity lhsT (slot-major => dst == partition). Self-loop rows arrive
by direct DMA and join the same PSUM accumulation. f32 accumulation.

Layer-2 table (h2 | a_s2 | a_d2, 256B rows) is assembled on-device with two
AllGather collectives (first half overlaps the tail of layer-1 compute); own
rows are read from the local shard at static offsets.

Feature order inside rows is (c,h)-interleaved (pos i = c*3+h) so the big
per-edge multiply has every operand at innermost stride 1.
"""
import numpy as np
import ml_dtypes

N = 50000
E = 800000
IN = 128
HID = 64
HEADS = 3
OUT = 64
NCORE = 8
P = 128
T = 49                              # tiles per core
CB = T * P + P                      # 6400 rows per core block (incl pad tile)
NTAB = NCORE * CB                   # 51200
SENTROW = 51072                     # block-7 pad-tile row (same local & global)
BASE = 32768
E1 = 256                            # L1 row elems (bf16) = 512B
E2 = 128                            # L2 row elems (bf16) = 256B
F1 = HEADS * HID                    # 192
NEG_SLOPE = 0.2
NQ = 4                              # SWDGE queues
GSZ = 4                             # tiles per gather group
XC = 7                              # dense1 tiles per chunk (49 = 7*7)
HALF_T = 25                         # tiles 0..24 -> AG region A, 25..48 -> B

bf16 = ml_dtypes.bfloat16

# interleaved feature order: row position i=(c*3+h) holds feature f=h*64+c
_POS = np.arange(F1)
PERM = (_POS % HEADS) * HID + (_POS // HEADS)


def _pack_idx(rows_flat):
    """rows_flat int64[nidx] (local table rows, nidx%128==0) -> int16
    [128, nidx//16] wrap-16 layout replicated across the 8 Q7 groups."""
    v = (rows_flat - BASE).astype(np.int16)
    cid = len(v) // 16
    block = v.reshape(cid, 16).T
    return np.tile(block, (8, 1))


def _g2row(c, t, p):
    """Global AG-table row for node position (c,t,p): two half regions."""
    t = np.asarray(t)
    return np.where(t < HALF_T,
                    c * 3200 + t * 128 + p,
                    25600 + c * 3200 + (t - HALF_T) * 128 + p)


def preprocess(edge_index):
    src = edge_index[0].astype(np.int64)
    dst = edge_index[1].astype(np.int64)
    deg = np.bincount(dst, minlength=N)            # in-degree, no self loops
    order = np.argsort(-deg, kind="stable")
    eorder = np.argsort(dst, kind="stable")
    esrc = src[eorder]
    eptr = np.zeros(N + 1, np.int64)
    eptr[1:] = np.cumsum(deg)

    # node placement: lowest-degree nodes reserved for partition 127
    n127 = T * NCORE
    main = order[:N - n127]
    tail = order[N - n127:]
    SENT = -1
    nid = np.full((NCORE, T, P), SENT, np.int64)
    MP = P - 1
    for t in range(T):
        blk = main[t * NCORE * MP: (t + 1) * NCORE * MP]
        for c in range(NCORE):
            sl = blk[c::NCORE]
            nid[c, t, :len(sl)] = sl
        tb = tail[t * NCORE: (t + 1) * NCORE]
        for c in range(NCORE):
            if c < len(tb):
                nid[c, t, P - 1] = tb[c]

    # position maps
    posc = np.zeros(N, np.int64)
    post = np.zeros(N, np.int64)
    posp = np.zeros(N, np.int64)
    for c in range(NCORE):
        for t in range(T):
            nn = nid[c, t]
            v = nn != SENT
            posc[nn[v]] = c
            post[nn[v]] = t
            posp[nn[v]] = np.nonzero(v)[0]

    # per-tile max degree (p0 holds each core's max; p127 handled below)
    dmax = np.zeros(T, np.int64)
    d127 = np.zeros(T, np.int64)
    for t in range(T):
        for c in range(NCORE):
            if nid[c, t, 0] != SENT:
                dmax[t] = max(dmax[t], deg[nid[c, t, 0]])
            if nid[c, t, P - 1] != SENT:
                d127[t] = max(d127[t], deg[nid[c, t, P - 1]])
    d_t = np.maximum(dmax, 1)

    # groups: greedy balance by slot count into ceil(T/GSZ) bins
    ng = (T + GSZ - 1) // GSZ
    while True:
        bins = [[] for _ in range(ng)]
        load = np.zeros(ng, np.int64)
        cnt = np.zeros(ng, np.int64)
        for t in np.argsort(-d_t, kind="stable"):
            elig = np.nonzero(cnt < GSZ)[0]
            b = elig[np.argmin(load[elig])]
            bins[b].append(int(t))
            load[b] += d_t[t]
            cnt[b] += 1
        groups = [sorted(b) for b in bins]
        # stripe boundaries per group (4 queue-striped calls)
        stripes = []
        bnd_local = {t: set() for t in range(T)}
        for g, mem in enumerate(groups):
            S = int(sum(d_t[t] for t in mem))
            bs = sorted(set(max(1, round(S * q / NQ)) for q in range(1, NQ + 1)))
            st = []
            s0 = 0
            for b in bs:
                if b > s0:
                    st.append((s0, b))
                    s0 = b
            stripes.append(st)
            # map stripe-end slots to (tile, local slot)
            offs = np.cumsum([0] + [int(d_t[t]) for t in mem])
            for (a, b) in st:
                e = b - 1
                mi = int(np.searchsorted(offs, e, side="right")) - 1
                bnd_local[mem[mi]].add(int(e - offs[mi]))
        # feasibility: p127 edges must fit in non-boundary slots
        ok = True
        for t in range(T):
            avail = int(d_t[t]) - len(bnd_local[t])
            if d127[t] > avail:
                d_t[t] += d127[t] - avail
                ok = False
        if ok:
            break

    # per-group slot offsets (for SBUF layout / idx columns)
    goff = []
    for mem in groups:
        offs = np.cumsum([0] + [int(d_t[t]) for t in mem])
        goff.append(offs)
    CE = int(d_t.sum()) * 8            # idx columns

    # index grids, both layers
    idx1 = np.zeros((NCORE, P, CE), np.int16)
    idx2 = np.zeros((NCORE, P, CE), np.int16)
    for c in range(NCORE):
        b = (posc - c) % NCORE
        lrow = b * CB + post * 128 + posp           # L1 local rotated rows
        grow = _g2row(posc, post, posp)             # L2 global AG rows
        blocks1, blocks2 = [], []
        for g, mem in enumerate(groups):
            S = int(goff[g][-1])
            r1 = np.full((S, P), SENTROW, np.int64)
            r2 = np.full((S, P), SENTROW, np.int64)
            for mi, t in enumerate(mem):
                off = int(goff[g][mi])
                d = int(d_t[t])
                p127_slots = [s for s in range(d) if s not in bnd_local[t]]
                for p in range(P):
                    n = nid[c, t, p]
                    if n < 0:
                        continue
                    k = int(deg[n])
                    ee = esrc[eptr[n]:eptr[n] + k]
                    if p == P - 1:
                        sl = np.array(p127_slots[:k], np.int64)
                    else:
                        sl = np.arange(k)
                    r1[off + sl, p] = lrow[ee]
                    r2[off + sl, p] = grow[ee]
            blocks1.append(_pack_idx(r1.reshape(-1)))
            blocks2.append(_pack_idx(r2.reshape(-1)))
        idx1[c] = np.concatenate(blocks1, axis=1)
        idx2[c] = np.concatenate(blocks2, axis=1)

    return dict(order=order, nid=nid, d_t=d_t, deg=deg, esrc=esrc, eptr=eptr,
                groups=groups, stripes=stripes, goff=goff,
                idx1=idx1, idx2=idx2, posc=posc, post=post, posp=posp)


def host_weights(x, W1, att_src1, att_dst1, b1, W2, att_src2, att_dst2, b2,
                 nid):
    W1s = np.stack([W1[:, h * HID:(h + 1) * HID] @ att_src1[h]
                    for h in range(HEADS)], 1)       # [128,3]
    W1d = np.stack([W1[:, h * HID:(h + 1) * HID] @ att_dst1[h]
                    for h in range(HEADS)], 1)
    W1e = np.concatenate([W1[:, PERM], W1s, W1d], axis=1)        # [128,198]
    W2e = np.concatenate([W2, (W2 @ att_src2[0])[:, None],
                          (W2 @ att_dst2[0])[:, None]], 1)       # [192,66]
    W2e = W2e[PERM, :]
    # compact global X^T in (c,t,p) order
    Xg = np.zeros((IN, NCORE * T * P), np.float32)
    for c in range(NCORE):
        for t in range(T):
            nn = nid[c, t]
            v = nn >= 0
            colbase = (c * T + t) * P
            Xg[:, colbase + np.nonzero(v)[0]] = x[nn[v]].T
    Xg = Xg.astype(bf16)
    xTs = []
    for c in range(NCORE):
        xTs.append(np.concatenate(
            [Xg[:, ((c + b) % NCORE) * T * P:(((c + b) % NCORE) + 1) * T * P]
             for b in range(NCORE)], axis=1))
    sent1 = np.zeros(E1, bf16)
    sent1.view(np.float32)[96:99] = -1e30
    sent2 = np.zeros(E2, bf16)
    sent2.view(np.float32)[32] = -1e30
    b1i = b1[PERM].astype(bf16)
    return dict(xTs=xTs, W1e=W1e.astype(bf16),
                W2e1=W2e[:128].astype(bf16), W2e2=W2e[128:].astype(bf16),
                sent1=sent1.reshape(1, E1), sent2=sent2.reshape(1, E2),
                b1i=np.tile(b1i, (P, 1)),
                b2b=np.tile(b2.astype(np.float32), (P, 1)))


# ---------------------------------------------------------------- emulation
def _bf(a):
    return a.astype(bf16).astype(np.float32)


def emulate(x, edge_index, W1, att_src1, att_dst1, b1, W2, att_src2, att_dst2,
            b2):
    pre = preprocess(edge_index)
    hw = host_weights(x, W1, att_src1, att_dst1, b1, W2, att_src2, att_dst2,
                      b2, pre["nid"])
    nid, d_t = pre["nid"], pre["d_t"]
    deg, esrc, eptr = pre["deg"], pre["esrc"], pre["eptr"]
    groups = pre["groups"]

    # dense1 (bf16 in, f32 psum)
    W1ef = hw["W1e"].astype(np.float32)
    b1f = hw["b1i"].astype(np.float32)[0]
    W2e1f = hw["W2e1"].astype(np.float32)
    W2e2f = hw["W2e2"].astype(np.float32)
    b2f = hw["b2b"][0]

    # table in GLOBAL node order (rotation only changes addressing)
    tab_h = np.zeros((N, F1), np.float32)
    tab_as = np.zeros((N, HEADS), np.float32)
    tab_ad = np.zeros((N, HEADS), np.float32)
    for c in range(NCORE):
        for t in range(T):
            nn = nid[c, t]
            v = nn >= 0
            xx = _bf(x[nn[v]])
            H = xx @ W1ef                      # f32 accum of bf16 inputs
            tab_h[nn[v]] = _bf(H[:, :F1])
            tab_as[nn[v]] = H[:, F1:F1 + 3]
            tab_ad[nn[v]] = H[:, F1 + 3:F1 + 6]

    tab2_h = np.zeros((N, OUT), np.float32)
    tab2_as = np.zeros((N, 1), np.float32)
    tab2_ad = np.zeros((N, 1), np.float32)

    def gat_layer(th, tas, tad, nf, layer):
        """th[N,nf] (perm'd for L1), tas/tad [N,H'] -> per-(c,t) outputs."""
        H_ = tas.shape[1]
        rep = nf // H_
        outs = np.zeros((NCORE, T, P, nf), np.float32)
        for c in range(NCORE):
            for t in range(T):
                d = int(d_t[t])
                rows = np.full((P, d), -1, np.int64)
                # boundary slots for p127
                g = next(gi for gi, mem in enumerate(groups) if t in mem)
                bset = set()
                offs = pre["goff"][g]
                for (a, bb) in pre["stripes"][g]:
                    e = bb - 1
                    mj = int(np.searchsorted(offs, e, side="right")) - 1
                    if groups[g][mj] == t:
                        bset.add(int(e - offs[mj]))
                p127_slots = [s for s in range(d) if s not in bset]
                for p in range(P):
                    n = nid[c, t, p]
                    if n < 0:
                        continue
                    k = int(deg[n])
                    sl = p127_slots[:k] if p == P - 1 else list(range(k))
                    rows[p, sl] = esrc[eptr[n]:eptr[n] + k]
                pad = rows < 0
                rr = np.where(pad, 0, rows)
                g_h = th[rr]                        # [P,d,nf] bf16-valued
                g_as = np.where(pad[:, :, None], -1e30, tas[rr])
                own = nid[c, t]
                ov = own >= 0
                oh = np.where(ov[:, None], th[np.where(ov, own, 0)], 0.0)
                oas = np.where(ov[:, None], tas[np.where(ov, own, 0)], -1e30)
                oad = np.where(ov[:, None], tad[np.where(ov, own, 0)], 0.0)
                # logits
                eE = g_as + oad[:, None, :]
                eE = np.maximum(eE, NEG_SLOPE * eE)
                wE = np.exp(eE)
                eS = oas + oad
                eS = np.maximum(eS, NEG_SLOPE * eS)
                wS = np.exp(eS)
                s = wE.sum(axis=1) + wS             # [P,H'] f32
                wEb = _bf(wE)
                wSb = _bf(wS)
                if layer == 1:
                    # interleaved: feature i=(cc*3+h) scaled by w[...,h]
                    wexp = np.repeat(wEb[:, :, None, :], HID, 2).reshape(P, d, nf)
                    sexp = np.repeat(wSb[:, None, :], HID, 1).reshape(P, nf)
                else:
                    wexp = np.repeat(wEb, rep, axis=2)
                    sexp = np.repeat(wSb, rep, axis=1)
                prod = _bf(g_h * wexp)
                prod[pad] = 0.0                     # pad rows gather sentinel
                sprod = _bf(oh * sexp)
                U = sprod.astype(np.float32) + prod.sum(axis=1, dtype=np.float32)
                r = _bf(1.0 / np.maximum(s, 1e-30))
                if layer == 1:
                    rexp = np.repeat(r[:, None, :], HID, 1).reshape(P, nf)
                    h1 = _bf(U * _bf(rexp))
                    h1 = _bf(h1 + b1f)
                    h1 = _bf(np.maximum(h1, 0) +
                             _bf(np.exp(np.minimum(h1, 0))) - 1)
                    outs[c, t] = h1
                else:
                    outs[c, t] = U * r + b2f
        return outs

    h1 = gat_layer(tab_h, tab_as, tab_ad, F1, 1)
    # dense2 (per tile, f32 accum of bf16)
    for c in range(NCORE):
        for t in range(T):
            o2 = _bf(h1[c, t]) @ np.concatenate([W2e1f, W2e2f], 0)
            nn = nid[c, t]
            v = nn >= 0
            tab2_h[nn[v]] = _bf(o2[v, :OUT])
            tab2_as[nn[v], 0] = o2[v, OUT]
            tab2_ad[nn[v], 0] = o2[v, OUT + 1]
    out = gat_layer(tab2_h, tab2_as, tab2_ad, OUT, 2)
    res = np.zeros((N, OUT), np.float32)
    for c in range(NCORE):
        for t in range(T):
            nn = nid[c, t]
            v = nn >= 0
            res[nn[v]] = out[c, t][v]
    return res


# ---------------------------------------------------------------- bass build
def _build(pre, pay1_bufs=3, pay2_bufs=4):
    import concourse.bacc as bacc
    import concourse.mybir as mybir
    import concourse.tile as tile
    from concourse.masks import make_identity

    d_t = pre["d_t"]
    groups, stripes, goff = pre["groups"], pre["stripes"], pre["goff"]
    CE = int(d_t.sum()) * 8
    MAXS = max(int(goff[g][-1]) for g in range(len(groups)))

    dt = mybir.dt
    nc = bacc.Bacc(num_devices=NCORE, num_swdge_queues=NQ)
    xT = nc.dram_tensor("xT", [IN, NCORE * T * P], dt.bfloat16, kind="ExternalInput")
    W1e = nc.dram_tensor("W1e", [IN, 198], dt.bfloat16, kind="ExternalInput")
    W2e1 = nc.dram_tensor("W2e1", [128, 66], dt.bfloat16, kind="ExternalInput")
    W2e2 = nc.dram_tensor("W2e2", [64, 66], dt.bfloat16, kind="ExternalInput")
    b1i = nc.dram_tensor("b1i", [P, F1], dt.bfloat16, kind="ExternalInput")
    b2b = nc.dram_tensor("b2b", [P, OUT], dt.float32, kind="ExternalInput")
    neg1 = nc.dram_tensor("neg1", [P, F1], dt.bfloat16, kind="ExternalInput")
    sent1 = nc.dram_tensor("sent1", [1, E1], dt.bfloat16, kind="ExternalInput")
    sent2 = nc.dram_tensor("sent2", [1, E2], dt.bfloat16, kind="ExternalInput")
    idx1 = nc.dram_tensor("idx1", [P, CE], dt.int16, kind="ExternalInput")
    idx2 = nc.dram_tensor("idx2", [P, CE], dt.int16, kind="ExternalInput")
    out2 = nc.dram_tensor("out2", [T * P, OUT], dt.float32, kind="ExternalOutput")
    tab1 = nc.dram_tensor("tab1", [NTAB, E1], dt.bfloat16)
    shard = nc.dram_tensor("shard", [CB, E2], dt.bfloat16)
    ag = nc.dram_tensor("ag", [NTAB, E2], dt.bfloat16, addr_space="Shared")

    with tile.TileContext(nc) as tc:
        with tc.tile_pool(name="const", bufs=1) as cp:
            w1_sb = cp.tile([IN, 198], dt.bfloat16)
            nc.sync.dma_start(out=w1_sb[:], in_=W1e[:, :])
            w2a_sb = cp.tile([128, 66], dt.bfloat16)
            nc.sync.dma_start(out=w2a_sb[:], in_=W2e1[:, :])
            w2b_sb = cp.tile([64, 66], dt.bfloat16)
            nc.sync.dma_start(out=w2b_sb[:], in_=W2e2[:, :])
            b1_sb = cp.tile([P, F1], dt.bfloat16)
            nc.sync.dma_start(out=b1_sb[:], in_=b1i[:, :])
            b2_sb = cp.tile([P, OUT], dt.float32)
            nc.sync.dma_start(out=b2_sb[:], in_=b2b[:, :])
            ide = cp.tile([P, P], dt.bfloat16)
            make_identity(nc, ide[:])
            neg1_sb = cp.tile([P, F1], dt.bfloat16)
            nc.sync.dma_start(out=neg1_sb[:], in_=neg1[:, :])
            i1_sb = cp.tile([P, CE], dt.int16)
            nc.sync.dma_start(out=i1_sb[:], in_=idx1[:, :])
            i2_sb = cp.tile([P, CE], dt.int16)
            nc.sync.dma_start(out=i2_sb[:], in_=idx2[:, :])
            sent1_sb = cp.tile([1, E1], dt.bfloat16)
            nc.sync.dma_start(out=sent1_sb[:], in_=sent1[:, :])
            nc.sync.dma_start(out=tab1[SENTROW:SENTROW + 1, :], in_=sent1_sb[:])
            sent2_sb = cp.tile([1, E2], dt.bfloat16)
            nc.sync.dma_start(out=sent2_sb[:], in_=sent2[:, :])
            nc.sync.dma_start(out=shard[T * P:T * P + 1, :], in_=sent2_sb[:])

            # ---------------- dense1: all 50176 rows, rotated layout
            with tc.tile_pool(name="xp", bufs=3) as xp, \
                 tc.tile_pool(name="rowp", bufs=3) as rowp, \
                 tc.tile_pool(name="psD", bufs=4, space="PSUM") as psD:
                for ch in range(NCORE * XC):
                    blk, j = divmod(ch, XC)
                    xch = xp.tile([P, XC * P], dt.bfloat16, tag="x")
                    nc.sync.dma_start(
                        out=xch[:], in_=xT[:, ch * XC * P:(ch + 1) * XC * P])
                    rt = rowp.tile([P, XC * E1], dt.bfloat16, tag="rt")
                    for k in range(XC):
                        pt = psD.tile([P, 198], dt.float32, tag="d1")
                        nc.tensor.matmul(pt[:], lhsT=xch[:, k * P:(k + 1) * P],
                                         rhs=w1_sb[:], start=True, stop=True)
                        if k % 2 == 0:
                            nc.scalar.activation(
                                rt[:, k * E1:k * E1 + F1], pt[:, :F1],
                                mybir.ActivationFunctionType.Copy)
                        else:
                            nc.vector.tensor_copy(
                                out=rt[:, k * E1:k * E1 + F1], in_=pt[:, :F1])
                        nc.vector.tensor_copy(
                            out=rt[:, k * E1 + F1:k * E1 + F1 + 12]
                                .bitcast(dt.float32),
                            in_=pt[:, F1:198])
                    dst = tab1[blk * CB + j * XC * P:
                               blk * CB + (j + 1) * XC * P, :]
                    nc.sync.dma_start(
                        out=dst.rearrange("(k p) e -> p k e", p=P),
                        in_=rt[:].rearrange("p (k e) -> p k e", e=E1))

            # ---------------- L1 edge phase
            tab_lo = tab1[BASE:, :]
            with tc.tile_pool(name="own", bufs=3) as ownp, \
                 tc.tile_pool(name="pay", bufs=pay1_bufs) as payp, \
                 tc.tile_pool(name="wp", bufs=3) as wp, \
                 tc.tile_pool(name="hp", bufs=3) as hp, \
                 tc.tile_pool(name="psU", bufs=3, space="PSUM") as psU, \
                 tc.tile_pool(name="psB", bufs=1, space="PSUM") as psB, \
                 tc.tile_pool(name="psO", bufs=2, space="PSUM") as psO:
                for g, mem in enumerate(groups):
                    M = len(mem)
                    S = int(goff[g][-1])
                    off_cols = int(np.sum([goff[gg][-1] for gg in range(g)]))
                    own = ownp.tile([P, GSZ * E1], dt.bfloat16, tag="own")
                    for mi, t in enumerate(mem):
                        nc.sync.dma_start(
                            out=own[:, mi * E1:(mi + 1) * E1],
                            in_=tab1[t * P:(t + 1) * P, :])
                    pay = payp.tile([P, MAXS * E1], dt.bfloat16, tag="pay")
                    for q, (s0, s1) in enumerate(stripes[g]):
                        nc.gpsimd.dma_gather(
                            out_ap=pay[:, s0 * E1:s1 * E1]
                                .rearrange("p (s e) -> p s e", e=E1),
                            in_ap=tab_lo,
                            idxs_ap=i1_sb[:, (off_cols + s0) * 8:
                                          (off_cols + s1) * 8],
                            num_idxs=(s1 - s0) * P,
                            num_idxs_reg=(s1 - s0) * P,
                            elem_size=E1, single_packet=False, queue_num=q)
                    ownf = own[:].bitcast(dt.float32)
                    ownv = ownf.rearrange("p (m e) -> p m e", e=128)
                    # self logits
                    eS = wp.tile([P, GSZ * 3], dt.float32, tag="eS")
                    nc.vector.tensor_tensor(
                        out=eS[:, :M * 3].rearrange("p (m h) -> p m h", h=3),
                        in0=ownv[:, :M, 96:99], in1=ownv[:, :M, 99:102],
                        op=mybir.AluOpType.add)
                    eS2 = wp.tile([P, GSZ * 3], dt.float32, tag="eS2")
                    nc.vector.scalar_tensor_tensor(
                        out=eS2[:, :M * 3], in0=eS[:, :M * 3], scalar=NEG_SLOPE,
                        in1=eS[:, :M * 3], op0=mybir.AluOpType.mult,
                        op1=mybir.AluOpType.max)
                    wS = wp.tile([P, GSZ * 3], dt.float32, tag="wS")
                    nc.scalar.activation(wS[:, :M * 3], eS2[:, :M * 3],
                                         mybir.ActivationFunctionType.Exp)
                    wSb = wp.tile([P, GSZ * 3], dt.bfloat16, tag="wSb")
                    nc.vector.tensor_copy(out=wSb[:, :M * 3], in_=wS[:, :M * 3])
                    # self multiply in place (interleaved (c,h))
                    oh = own[:].rearrange("p (m e) -> p m e", e=E1)[:, :M, :F1] \
                        .rearrange("p m (c h) -> p m c h", h=3)
                    nc.vector.tensor_tensor(
                        out=oh,
                        in0=oh,
                        in1=wSb[:, :M * 3].rearrange("p (m h) -> p m h", h=3)
                            .unsqueeze(2).to_broadcast([P, M, HID, 3]),
                        op=mybir.AluOpType.mult)
                    # edge logits (per-member add, group-wide rest)
                    et = wp.tile([P, MAXS * 3], dt.float32, tag="et")
                    payf = pay[:].bitcast(dt.float32)
                    a_s = payf.rearrange("p (s e) -> p s e", e=128)
                    for mi, t in enumerate(mem):
                        o0, o1 = int(goff[g][mi]), int(goff[g][mi + 1])
                        nc.vector.tensor_tensor(
                            out=et[:, o0 * 3:o1 * 3]
                                .rearrange("p (s h) -> p s h", h=3),
                            in0=a_s[:, o0:o1, 96:99],
                            in1=ownv[:, mi, 99:102].unsqueeze(1)
                                .to_broadcast([P, o1 - o0, 3]),
                            op=mybir.AluOpType.add)
                    et2 = wp.tile([P, MAXS * 3], dt.float32, tag="et2")
                    nc.vector.scalar_tensor_tensor(
                        out=et2[:, :S * 3], in0=et[:, :S * 3], scalar=NEG_SLOPE,
                        in1=et[:, :S * 3], op0=mybir.AluOpType.mult,
                        op1=mybir.AluOpType.max)
                    wf = wp.tile([P, MAXS * 3], dt.float32, tag="wf")
                    nc.scalar.activation(wf[:, :S * 3], et2[:, :S * 3],
                                         mybir.ActivationFunctionType.Exp)
                    wb = wp.tile([P, MAXS * 3], dt.bfloat16, tag="wb")
                    nc.vector.tensor_copy(out=wb[:, :S * 3], in_=wf[:, :S * 3])
                    # big multiply in place
                    h_view = pay[:].rearrange("p (s e) -> p s e", e=E1)[:, :S, :F1] \
                        .rearrange("p s (c h) -> p s c h", h=3)
                    w_view = wb[:, :S * 3].rearrange("p (s h) -> p s h", h=3) \
                        .unsqueeze(2).to_broadcast([P, S, HID, 3])
                    nc.vector.tensor_tensor(out=h_view, in0=h_view, in1=w_view,
                                            op=mybir.AluOpType.mult)
                    # denominators (per member) + self
                    st = wp.tile([P, GSZ * 3], dt.float32, tag="st")
                    for mi, t in enumerate(mem):
                        o0, o1 = int(goff[g][mi]), int(goff[g][mi + 1])
                        nc.vector.tensor_reduce(
                            out=st[:, mi * 3:(mi + 1) * 3],
                            in_=wf[:, o0 * 3:o1 * 3]
                                .rearrange("p (s h) -> p h s", h=3),
                            axis=mybir.AxisListType.X, op=mybir.AluOpType.add)
                    nc.vector.tensor_add(out=st[:, :M * 3], in0=st[:, :M * 3],
                                         in1=wS[:, :M * 3])
                    nc.vector.tensor_scalar_max(out=st[:, :M * 3],
                                                in0=st[:, :M * 3], scalar1=1e-30)
                    rc = wp.tile([P, GSZ * 3], dt.float32, tag="rc")
                    nc.vector.reciprocal(out=rc[:, :M * 3], in_=st[:, :M * 3])
                    rcb = wp.tile([P, GSZ * 3], dt.bfloat16, tag="rcb")
                    nc.vector.tensor_copy(out=rcb[:, :M * 3], in_=rc[:, :M * 3])
                    # accumulate + psum copy per member
                    h1g = hp.tile([P, GSZ * F1], dt.bfloat16, tag="h1g")
                    for mi, t in enumerate(mem):
                        o0 = int(goff[g][mi])
                        d = int(d_t[t])
                        U = psU.tile([P, F1], dt.float32, tag="U")
                        nc.tensor.matmul(U[:], lhsT=ide[:],
                                         rhs=own[:, mi * E1:mi * E1 + F1],
                                         start=True, stop=False)
                        for s in range(d):
                            nc.tensor.matmul(
                                U[:], lhsT=ide[:],
                                rhs=pay[:, (o0 + s) * E1:(o0 + s) * E1 + F1],
                                start=False, stop=(s == d - 1))
                        nc.scalar.activation(h1g[:, mi * F1:(mi + 1) * F1],
                                             U[:],
                                             mybir.ActivationFunctionType.Copy)
                    # group-wide normalize + bias + ELU
                    h1v = h1g[:, :M * F1].rearrange("p (m c h) -> p m c h", h=3, c=HID)
                    nc.vector.tensor_tensor(
                        out=h1v, in0=h1v,
                        in1=rcb[:, :M * 3].rearrange("p (m h) -> p m h", h=3)
                            .unsqueeze(2).to_broadcast([P, M, HID, 3]),
                        op=mybir.AluOpType.mult)
                    nc.vector.tensor_tensor(
                        out=h1g[:, :M * F1].rearrange("p (m f) -> p m f", f=F1),
                        in0=h1g[:, :M * F1].rearrange("p (m f) -> p m f", f=F1),
                        in1=b1_sb[:].unsqueeze(1).to_broadcast([P, M, F1]),
                        op=mybir.AluOpType.add)
                    tmin = hp.tile([P, GSZ * F1], dt.bfloat16, tag="tmin")
                    nc.vector.scalar_tensor_tensor(
                        out=tmin[:, :M * F1], in0=h1g[:, :M * F1], scalar=0.0,
                        in1=h1g[:, :M * F1], op0=mybir.AluOpType.mult,
                        op1=mybir.AluOpType.min)
                    texp = hp.tile([P, GSZ * F1], dt.bfloat16, tag="texp")
                    nc.scalar.activation(texp[:, :M * F1], tmin[:, :M * F1],
                                         mybir.ActivationFunctionType.Exp)
                    nc.vector.scalar_tensor_tensor(
                        out=h1g[:, :M * F1], in0=h1g[:, :M * F1], scalar=0.0,
                        in1=texp[:, :M * F1], op0=mybir.AluOpType.max,
                        op1=mybir.AluOpType.add)
                    nc.vector.tensor_tensor(
                        out=h1g[:, :M * F1].rearrange("p (m f) -> p m f", f=F1),
                        in0=h1g[:, :M * F1].rearrange("p (m f) -> p m f", f=F1),
                        in1=neg1_sb[:].unsqueeze(1).to_broadcast([P, M, F1]),
                        op=mybir.AluOpType.add)
                    # dense2 per member
                    for mi, t in enumerate(mem):
                        tp1 = psB.tile([P, P], dt.bfloat16, tag="tp1")
                        nc.tensor.transpose(tp1[:], h1g[:, mi * F1:mi * F1 + P],
                                            ide[:])
                        tp2 = psB.tile([64, P], dt.bfloat16, tag="tp2")
                        nc.tensor.transpose(tp2[:],
                                            h1g[:, mi * F1 + P:(mi + 1) * F1],
                                            ide[:])
                        hT1 = hp.tile([P, P], dt.bfloat16, tag="hT1")
                        nc.vector.tensor_copy(out=hT1[:], in_=tp1[:])
                        hT2 = hp.tile([64, P], dt.bfloat16, tag="hT2")
                        nc.vector.tensor_copy(out=hT2[:], in_=tp2[:])
                        o2 = psO.tile([P, 66], dt.float32, tag="o2")
                        nc.tensor.matmul(o2[:], lhsT=hT1[:], rhs=w2a_sb[:],
                                         start=True, stop=False)
                        nc.tensor.matmul(o2[:], lhsT=hT2[:], rhs=w2b_sb[:],
                                         start=False, stop=True)
                        r2 = hp.tile([P, 68], dt.bfloat16, tag="r2")
                        nc.scalar.activation(r2[:, :OUT], o2[:, :OUT],
                                             mybir.ActivationFunctionType.Copy)
                        nc.vector.tensor_copy(
                            out=r2[:, OUT:OUT + 4].bitcast(dt.float32),
                            in_=o2[:, OUT:OUT + 2])
                        nc.sync.dma_start(out=shard[t * P:(t + 1) * P, :68],
                                          in_=r2[:])

            # ---------------- AllGather table2 (two halves)
            nc.gpsimd.collective_compute(
                "AllGather", mybir.AluOpType.bypass,
                replica_groups=[list(range(NCORE))],
                ins=[shard[0:HALF_T * P, :].opt()],
                outs=[ag[0:NCORE * HALF_T * P, :].opt()])
            nc.gpsimd.collective_compute(
                "AllGather", mybir.AluOpType.bypass,
                replica_groups=[list(range(NCORE))],
                ins=[shard[HALF_T * P:CB, :].opt()],
                outs=[ag[NCORE * HALF_T * P:NTAB, :].opt()])

            # ---------------- L2 edge phase
            ag_lo = ag[BASE:, :]
            with tc.tile_pool(name="own2", bufs=3) as ownp2, \
                 tc.tile_pool(name="pay2", bufs=pay2_bufs) as payp2, \
                 tc.tile_pool(name="wp2", bufs=3) as wp2, \
                 tc.tile_pool(name="op2", bufs=3) as op2, \
                 tc.tile_pool(name="psU2", bufs=4, space="PSUM") as psU2:
                for g, mem in enumerate(groups):
                    M = len(mem)
                    S = int(goff[g][-1])
                    off_cols = int(np.sum([goff[gg][-1] for gg in range(g)]))
                    own = ownp2.tile([P, GSZ * E2], dt.bfloat16, tag="own")
                    for mi, t in enumerate(mem):
                        nc.sync.dma_start(
                            out=own[:, mi * E2:(mi + 1) * E2],
                            in_=shard[t * P:(t + 1) * P, :])
                    pay = payp2.tile([P, MAXS * E2], dt.bfloat16, tag="pay")
                    for q, (s0, s1) in enumerate(stripes[g]):
                        nc.gpsimd.dma_gather(
                            out_ap=pay[:, s0 * E2:s1 * E2]
                                .rearrange("p (s e) -> p s e", e=E2),
                            in_ap=ag_lo,
                            idxs_ap=i2_sb[:, (off_cols + s0) * 8:
                                          (off_cols + s1) * 8],
                            num_idxs=(s1 - s0) * P,
                            num_idxs_reg=(s1 - s0) * P,
                            elem_size=E2, single_packet=False, queue_num=q)
                    ownf = own[:].bitcast(dt.float32)
                    ownv = ownf.rearrange("p (m e) -> p m e", e=64)
                    eS = wp2.tile([P, GSZ], dt.float32, tag="eS")
                    nc.vector.tensor_tensor(
                        out=eS[:, :M].unsqueeze(2),
                        in0=ownv[:, :M, 32:33], in1=ownv[:, :M, 33:34],
                        op=mybir.AluOpType.add)
                    eS2 = wp2.tile([P, GSZ], dt.float32, tag="eS2")
                    nc.vector.scalar_tensor_tensor(
                        out=eS2[:, :M], in0=eS[:, :M], scalar=NEG_SLOPE,
                        in1=eS[:, :M], op0=mybir.AluOpType.mult,
                        op1=mybir.AluOpType.max)
                    wS = wp2.tile([P, GSZ], dt.float32, tag="wS")
                    nc.scalar.activation(wS[:, :M], eS2[:, :M],
                                         mybir.ActivationFunctionType.Exp)
                    wSb = wp2.tile([P, GSZ], dt.bfloat16, tag="wSb")
                    nc.vector.tensor_copy(out=wSb[:, :M], in_=wS[:, :M])
                    oh = own[:].rearrange("p (m e) -> p m e", e=E2)[:, :M, :OUT]
                    nc.vector.tensor_tensor(
                        out=oh, in0=oh,
                        in1=wSb[:, :M].unsqueeze(2).to_broadcast([P, M, OUT]),
                        op=mybir.AluOpType.mult)
                    et = wp2.tile([P, MAXS], dt.float32, tag="et")
                    payf = pay[:].bitcast(dt.float32)
                    a_s = payf.rearrange("p (s e) -> p s e", e=64)
                    for mi, t in enumerate(mem):
                        o0, o1 = int(goff[g][mi]), int(goff[g][mi + 1])
                        nc.vector.tensor_tensor(
                            out=et[:, o0:o1].unsqueeze(2),
                            in0=a_s[:, o0:o1, 32:33],
                            in1=ownv[:, mi, 33:34].unsqueeze(1)
                                .to_broadcast([P, o1 - o0, 1]),
                            op=mybir.AluOpType.add)
                    et2 = wp2.tile([P, MAXS], dt.float32, tag="et2")
                    nc.vector.scalar_tensor_tensor(
                        out=et2[:, :S], in0=et[:, :S], scalar=NEG_SLOPE,
                        in1=et[:, :S], op0=mybir.AluOpType.mult,
                        op1=mybir.AluOpType.max)
                    wf = wp2.tile([P, MAXS], dt.float32, tag="wf")
                    nc.scalar.activation(wf[:, :S], et2[:, :S],
                                         mybir.ActivationFunctionType.Exp)
                    wb = wp2.tile([P, MAXS], dt.bfloat16, tag="wb")
                    nc.vector.tensor_copy(out=wb[:, :S], in_=wf[:, :S])
                    h_view = pay[:].rearrange("p (s e) -> p s e", e=E2)[:, :S, :OUT]
                    w_view = wb[:, :S].unsqueeze(2).to_broadcast([P, S, OUT])
                    nc.vector.tensor_tensor(out=h_view, in0=h_view, in1=w_view,
                                            op=mybir.AluOpType.mult)
                    st = wp2.tile([P, GSZ], dt.float32, tag="st")
                    for mi, t in enumerate(mem):
                        o0, o1 = int(goff[g][mi]), int(goff[g][mi + 1])
                        nc.vector.tensor_reduce(
                            out=st[:, mi:mi + 1], in_=wf[:, o0:o1],
                            axis=mybir.AxisListType.X, op=mybir.AluOpType.add)
                    nc.vector.tensor_add(out=st[:, :M], in0=st[:, :M],
                                         in1=wS[:, :M])
                    nc.vector.tensor_scalar_max(out=st[:, :M], in0=st[:, :M],
                                                scalar1=1e-30)
                    rc = wp2.tile([P, GSZ], dt.float32, tag="rc")
                    nc.vector.reciprocal(out=rc[:, :M], in_=st[:, :M])
                    for mi, t in enumerate(mem):
                        o0 = int(goff[g][mi])
                        d = int(d_t[t])
                        U = psU2.tile([P, OUT], dt.float32, tag="U2")
                        nc.tensor.matmul(U[:], lhsT=ide[:],
                                         rhs=own[:, mi * E2:mi * E2 + OUT],
                                         start=True, stop=False)
                        for s in range(d):
                            nc.tensor.matmul(
                                U[:], lhsT=ide[:],
                                rhs=pay[:, (o0 + s) * E2:(o0 + s) * E2 + OUT],
                                start=False, stop=(s == d - 1))
                        ot = op2.tile([P, OUT], dt.float32, tag="ot")
                        nc.scalar.activation(ot[:], U[:],
                                             mybir.ActivationFunctionType.Copy)
                        nc.vector.tensor_tensor(
                            out=ot[:], in0=ot[:],
                            in1=rc[:, mi:mi + 1].to_broadcast([P, OUT]),
                            op=mybir.AluOpType.mult)
                        nc.vector.tensor_add(out=ot[:], in0=ot[:], in1=b2_sb[:])
                        nc.sync.dma_start(out=out2[t * P:(t + 1) * P, :],
                                          in_=ot[:])
    nc.compile()
    return nc


# ---------------------------------------------------------------- kernel
def kernel(x, edge_index, W1, att_src1, att_dst1, b1, W2, att_src2, att_dst2,
           b2, _emulate=False, _timing=None):
    x = np.asarray(x, np.float32)
    edge_index = np.asarray(edge_index)
    W1 = np.asarray(W1, np.float32)
    att_src1 = np.asarray(att_src1, np.float32)
    att_dst1 = np.asarray(att_dst1, np.float32)
    b1 = np.asarray(b1, np.float32)
    W2 = np.asarray(W2, np.float32)
    att_src2 = np.asarray(att_src2, np.float32)
    att_dst2 = np.asarray(att_dst2, np.float32)
    b2 = np.asarray(b2, np.float32)

    if _emulate:
        return emulate(x, edge_index, W1, att_src1, att_dst1, b1,
                       W2, att_src2, att_dst2, b2)

    from concourse.bass_utils import run_bass_kernel_spmd
    import time as _time

    pre = preprocess(edge_index)
    hw = host_weights(x, W1, att_src1, att_dst1, b1, W2, att_src2, att_dst2,
                      b2, pre["nid"])
    nc = _build(pre)
    neg1 = np.full((P, F1), -1.0, bf16)
    maps = [dict(xT=hw["xTs"][c], W1e=hw["W1e"], W2e1=hw["W2e1"],
                 W2e2=hw["W2e2"], b1i=hw["b1i"], b2b=hw["b2b"],
                 neg1=neg1, sent1=hw["sent1"], sent2=hw["sent2"],
                 idx1=pre["idx1"][c], idx2=pre["idx2"][c])
            for c in range(NCORE)]

    trace = _timing is not None
    res = None
    for attempt in range(3):
        try:
            res = run_bass_kernel_spmd(nc, maps, core_ids=list(range(NCORE)),
                                       trace=trace and attempt == 0)
            break
        except Exception:
            if attempt == 2:
                raise
            _time.sleep(45)

    nid = pre["nid"]
    out = np.zeros((N, OUT), np.float32)
    for c in range(NCORE):
        o = res.results[c]["out2"]
        nn = nid[c].reshape(-1)
        valid = nn >= 0
        out[nn[valid]] = o[valid]

    if _timing is not None:
        _timing["neff1_ns"] = res.exec_time_ns
        _timing["neff2_ns"] = 0
    return out


# revision 10
# speedup vs baseline: 1.0744x; 1.0085x over previous
"""2-layer GAT (PyG-style) on TRN2, 8 NeuronCores — single-NEFF version.

Strategy: nodes sorted by in-degree (self-loops excluded) and dealt
round-robin across the 8 cores into 49 tiles/core of 128 nodes. Table rows
live in DRAM in (core, tile, partition) order, ROTATED per core so each
core's own tiles are block 0 (static offsets for self-loop rows and per-tile
a_d loads). Per-edge source rows are fetched with dma_gather in slot-major
order (dst node == partition), striped across the 4 SWDGE queues per
tile-group. Self-loop rows and per-tile a_d come by direct DMA (no gather).

Aggregation: DVE multiplies gathered rows by edge weights in place, then the
segment-sum runs on the TensorEngine as PSUM-accumulating matmuls with a
static identity lhsT (slot-major => dst == partition). Self-loop rows arrive
by direct DMA and join the same PSUM accumulation. f32 accumulation.

Layer-2 table (h2 | a_s2 | a_d2, 256B rows) is assembled on-device with two
AllGather collectives (first half overlaps the tail of layer-1 compute); own
rows are read from the local shard at static offsets.

Feature order inside rows is (c,h)-interleaved (pos i = c*3+h) so the big
per-edge multiply has every operand at innermost stride 1.
"""
import numpy as np
import ml_dtypes

N = 50000
E = 800000
IN = 128
HID = 64
HEADS = 3
OUT = 64
NCORE = 8
P = 128
T = 49                              # tiles per core
CB = T * P + P                      # 6400 rows per core block (incl pad tile)
NTAB = NCORE * CB                   # 51200
SENTROW = 51072                     # block-7 pad-tile row (same local & global)
BASE = 32768
E1 = 256                            # L1 row elems (bf16) = 512B
E2 = 128                            # L2 row elems (bf16) = 256B
F1 = HEADS * HID                    # 192
NEG_SLOPE = 0.2
NQ = 4                              # SWDGE queues
GSZ = 4                             # tiles per gather group
XC = 7                              # dense1 tiles per chunk (49 = 7*7)
HALF_T = 25                         # tiles 0..24 -> AG region A, 25..48 -> B

bf16 = ml_dtypes.bfloat16

# interleaved feature order: row position i=(c*3+h) holds feature f=h*64+c
_POS = np.arange(F1)
PERM = (_POS % HEADS) * HID + (_POS // HEADS)


def _pack_idx(rows_flat):
    """rows_flat int64[nidx] (local table rows, nidx%128==0) -> int16
    [128, nidx//16] wrap-16 layout replicated across the 8 Q7 groups."""
    v = (rows_flat - BASE).astype(np.int16)
    cid = len(v) // 16
    block = v.reshape(cid, 16).T
    return np.tile(block, (8, 1))


def _g2row(c, t, p):
    """Global AG-table row for node position (c,t,p): two half regions."""
    t = np.asarray(t)
    return np.where(t < HALF_T,
                    c * 3200 + t * 128 + p,
                    25600 + c * 3200 + (t - HALF_T) * 128 + p)


def preprocess(edge_index):
    src = edge_index[0].astype(np.int64)
    dst = edge_index[1].astype(np.int64)
    deg = np.bincount(dst, minlength=N)            # in-degree, no self loops
    order = np.argsort(-deg, kind="stable")
    eorder = np.argsort(dst, kind="stable")
    esrc = src[eorder]
    eptr = np.zeros(N + 1, np.int64)
    eptr[1:] = np.cumsum(deg)

    # node placement: lowest-degree nodes reserved for partition 127
    n127 = T * NCORE
    main = order[:N - n127]
    tail = order[N - n127:]
    SENT = -1
    nid = np.full((NCORE, T, P), SENT, np.int64)
    MP = P - 1
    for t in range(T):
        blk = main[t * NCORE * MP: (t + 1) * NCORE * MP]
        for c in range(NCORE):
            sl = blk[c::NCORE]
            nid[c, t, :len(sl)] = sl
        tb = tail[t * NCORE: (t + 1) * NCORE]
        for c in range(NCORE):
            if c < len(tb):
                nid[c, t, P - 1] = tb[c]

    # position maps
    posc = np.zeros(N, np.int64)
    post = np.zeros(N, np.int64)
    posp = np.zeros(N, np.int64)
    for c in range(NCORE):
        for t in range(T):
            nn = nid[c, t]
            v = nn != SENT
            posc[nn[v]] = c
            post[nn[v]] = t
            posp[nn[v]] = np.nonzero(v)[0]

    # per-tile max degree (p0 holds each core's max; p127 handled below)
    dmax = np.zeros(T, np.int64)
    d127 = np.zeros(T, np.int64)
    for t in range(T):
        for c in range(NCORE):
            if nid[c, t, 0] != SENT:
                dmax[t] = max(dmax[t], deg[nid[c, t, 0]])
            if nid[c, t, P - 1] != SENT:
                d127[t] = max(d127[t], deg[nid[c, t, P - 1]])
    d_t = np.maximum(dmax, 1)

    # groups: greedy balance by slot count into ceil(T/GSZ) bins
    ng = (T + GSZ - 1) // GSZ
    while True:
        bins = [[] for _ in range(ng)]
        load = np.zeros(ng, np.int64)
        cnt = np.zeros(ng, np.int64)
        for t in np.argsort(-d_t, kind="stable"):
            elig = np.nonzero(cnt < GSZ)[0]
            b = elig[np.argmin(load[elig])]
            bins[b].append(int(t))
            load[b] += d_t[t]
            cnt[b] += 1
        groups = [sorted(b) for b in bins]
        # stripe boundaries per group (4 queue-striped calls)
        stripes = []
        bnd_local = {t: set() for t in range(T)}
        for g, mem in enumerate(groups):
            S = int(sum(d_t[t] for t in mem))
            bs = sorted(set(max(1, round(S * q / NQ)) for q in range(1, NQ + 1)))
            st = []
            s0 = 0
            for b in bs:
                if b > s0:
                    st.append((s0, b))
                    s0 = b
            stripes.append(st)
            # map stripe-end slots to (tile, local slot)
            offs = np.cumsum([0] + [int(d_t[t]) for t in mem])
            for (a, b) in st:
                e = b - 1
                mi = int(np.searchsorted(offs, e, side="right")) - 1
                bnd_local[mem[mi]].add(int(e - offs[mi]))
        # feasibility: p127 edges must fit in non-boundary slots
        ok = True
        for t in range(T):
            avail = int(d_t[t]) - len(bnd_local[t])
            if d127[t] > avail:
                d_t[t] += d127[t] - avail
                ok = False
        if ok:
            break

    # per-group slot offsets (for SBUF layout / idx columns)
    goff = []
    for mem in groups:
        offs = np.cumsum([0] + [int(d_t[t]) for t in mem])
        goff.append(offs)
    CE = int(d_t.sum()) * 8            # idx columns

    # index grids, both layers
    idx1 = np.zeros((NCORE, P, CE), np.int16)
    idx2 = np.zeros((NCORE, P, CE), np.int16)
    for c in range(NCORE):
        b = (posc - c) % NCORE
        lrow = b * CB + post * 128 + posp           # L1 local rotated rows
        grow = _g2row(posc, post, posp)             # L2 global AG rows
        blocks1, blocks2 = [], []
        for g, mem in enumerate(groups):
            S = int(goff[g][-1])
            r1 = np.full((S, P), SENTROW, np.int64)
            r2 = np.full((S, P), SENTROW, np.int64)
            for mi, t in enumerate(mem):
                off = int(goff[g][mi])
                d = int(d_t[t])
                p127_slots = [s for s in range(d) if s not in bnd_local[t]]
                for p in range(P):
                    n = nid[c, t, p]
                    if n < 0:
                        continue
                    k = int(deg[n])
                    ee = esrc[eptr[n]:eptr[n] + k]
                    if p == P - 1:
                        sl = np.array(p127_slots[:k], np.int64)
                    else:
                        sl = np.arange(k)
                    r1[off + sl, p] = lrow[ee]
                    r2[off + sl, p] = grow[ee]
            blocks1.append(_pack_idx(r1.reshape(-1)))
            blocks2.append(_pack_idx(r2.reshape(-1)))
        idx1[c] = np.concatenate(blocks1, axis=1)
        idx2[c] = np.concatenate(blocks2, axis=1)

    return dict(order=order, nid=nid, d_t=d_t, deg=deg, esrc=esrc, eptr=eptr,
                groups=groups, stripes=stripes, goff=goff,
                idx1=idx1, idx2=idx2, posc=posc, post=post, posp=posp)


def host_weights(x, W1, att_src1, att_dst1, b1, W2, att_src2, att_dst2, b2,
                 nid):
    W1s = np.stack([W1[:, h * HID:(h + 1) * HID] @ att_src1[h]
                    for h in range(HEADS)], 1)       # [128,3]
    W1d = np.stack([W1[:, h * HID:(h + 1) * HID] @ att_dst1[h]
                    for h in range(HEADS)], 1)
    W1e = np.concatenate([W1[:, PERM], W1s, W1d], axis=1)        # [128,198]
    W2e = np.concatenate([W2, (W2 @ att_src2[0])[:, None],
                          (W2 @ att_dst2[0])[:, None]], 1)       # [192,66]
    W2e = W2e[PERM, :]
    # compact global X^T in (c,t,p) order
    Xg = np.zeros((IN, NCORE * T * P), np.float32)
    for c in range(NCORE):
        for t in range(T):
            nn = nid[c, t]
            v = nn >= 0
            colbase = (c * T + t) * P
            Xg[:, colbase + np.nonzero(v)[0]] = x[nn[v]].T
    Xg = Xg.astype(bf16)
    xTs = []
    for c in range(NCORE):
        xTs.append(np.concatenate(
            [Xg[:, ((c + b) % NCORE) * T * P:(((c + b) % NCORE) + 1) * T * P]
             for b in range(NCORE)], axis=1))
    sent1 = np.zeros(E1, bf16)
    sent1.view(np.float32)[96:99] = -1e30
    sent2 = np.zeros(E2, bf16)
    sent2.view(np.float32)[32] = -1e30
    b1i = b1[PERM].astype(bf16)
    return dict(xTs=xTs, W1e=W1e.astype(bf16),
                W2e1=W2e[:128].astype(bf16), W2e2=W2e[128:].astype(bf16),
                sent1=sent1.reshape(1, E1), sent2=sent2.reshape(1, E2),
                b1i=np.tile(b1i, (P, 1)),
                b2b=np.tile(b2.astype(np.float32), (P, 1)))


# ---------------------------------------------------------------- emulation
def _bf(a):
    return a.astype(bf16).astype(np.float32)


def emulate(x, edge_index, W1, att_src1, att_dst1, b1, W2, att_src2, att_dst2,
            b2):
    pre = preprocess(edge_index)
    hw = host_weights(x, W1, att_src1, att_dst1, b1, W2, att_src2, att_dst2,
                      b2, pre["nid"])
    nid, d_t = pre["nid"], pre["d_t"]
    deg, esrc, eptr = pre["deg"], pre["esrc"], pre["eptr"]
    groups = pre["groups"]

    # dense1 (bf16 in, f32 psum)
    W1ef = hw["W1e"].astype(np.float32)
    b1f = hw["b1i"].astype(np.float32)[0]
    W2e1f = hw["W2e1"].astype(np.float32)
    W2e2f = hw["W2e2"].astype(np.float32)
    b2f = hw["b2b"][0]

    # table in GLOBAL node order (rotation only changes addressing)
    tab_h = np.zeros((N, F1), np.float32)
    tab_as = np.zeros((N, HEADS), np.float32)
    tab_ad = np.zeros((N, HEADS), np.float32)
    for c in range(NCORE):
        for t in range(T):
            nn = nid[c, t]
            v = nn >= 0
            xx = _bf(x[nn[v]])
            H = xx @ W1ef                      # f32 accum of bf16 inputs
            tab_h[nn[v]] = _bf(H[:, :F1])
            tab_as[nn[v]] = H[:, F1:F1 + 3]
            tab_ad[nn[v]] = H[:, F1 + 3:F1 + 6]

    tab2_h = np.zeros((N, OUT), np.float32)
    tab2_as = np.zeros((N, 1), np.float32)
    tab2_ad = np.zeros((N, 1), np.float32)

    def gat_layer(th, tas, tad, nf, layer):
        """th[N,nf] (perm'd for L1), tas/tad [N,H'] -> per-(c,t) outputs."""
        H_ = tas.shape[1]
        rep = nf // H_
        outs = np.zeros((NCORE, T, P, nf), np.float32)
        for c in range(NCORE):
            for t in range(T):
                d = int(d_t[t])
                rows = np.full((P, d), -1, np.int64)
                # boundary slots for p127
                g = next(gi for gi, mem in enumerate(groups) if t in mem)
                bset = set()
                offs = pre["goff"][g]
                for (a, bb) in pre["stripes"][g]:
                    e = bb - 1
                    mj = int(np.searchsorted(offs, e, side="right")) - 1
                    if groups[g][mj] == t:
                        bset.add(int(e - offs[mj]))
                p127_slots = [s for s in range(d) if s not in bset]
                for p in range(P):
                    n = nid[c, t, p]
                    if n < 0:
                        continue
                    k = int(deg[n])
                    sl = p127_slots[:k] if p == P - 1 else list(range(k))
                    rows[p, sl] = esrc[eptr[n]:eptr[n] + k]
                pad = rows < 0
                rr = np.where(pad, 0, rows)
                g_h = th[rr]                        # [P,d,nf] bf16-valued
                g_as = np.where(pad[:, :, None], -1e30, tas[rr])
                own = nid[c, t]
                ov = own >= 0
                oh = np.where(ov[:, None], th[np.where(ov, own, 0)], 0.0)
                oas = np.where(ov[:, None], tas[np.where(ov, own, 0)], -1e30)
                oad = np.where(ov[:, None], tad[np.where(ov, own, 0)], 0.0)
                # logits
                eE = g_as + oad[:, None, :]
                eE = np.maximum(eE, NEG_SLOPE * eE)
                wE = np.exp(eE)
                eS = oas + oad
                eS = np.maximum(eS, NEG_SLOPE * eS)
                wS = np.exp(eS)
                s = wE.sum(axis=1) + wS             # [P,H'] f32
                wEb = _bf(wE)
                wSb = _bf(wS)
                if layer == 1:
                    # interleaved: feature i=(cc*3+h) scaled by w[...,h]
                    wexp = np.repeat(wEb[:, :, None, :], HID, 2).reshape(P, d, nf)
                    sexp = np.repeat(wSb[:, None, :], HID, 1).reshape(P, nf)
                else:
                    wexp = np.repeat(wEb, rep, axis=2)
                    sexp = np.repeat(wSb, rep, axis=1)
                prod = _bf(g_h * wexp)
                prod[pad] = 0.0                     # pad rows gather sentinel
                sprod = _bf(oh * sexp)
                U = sprod.astype(np.float32) + prod.sum(axis=1, dtype=np.float32)
                r = _bf(1.0 / np.maximum(s, 1e-30))
                if layer == 1:
                    rexp = np.repeat(r[:, None, :], HID, 1).reshape(P, nf)
                    h1 = _bf(U * _bf(rexp))
                    h1 = _bf(h1 + b1f)
                    h1 = _bf(np.maximum(h1, 0) +
                             _bf(np.exp(np.minimum(h1, 0))) - 1)
                    outs[c, t] = h1
                else:
                    outs[c, t] = U * r + b2f
        return outs

    h1 = gat_layer(tab_h, tab_as, tab_ad, F1, 1)
    # dense2 (per tile, f32 accum of bf16)
    for c in range(NCORE):
        for t in range(T):
            o2 = _bf(h1[c, t]) @ np.concatenate([W2e1f, W2e2f], 0)
            nn = nid[c, t]
            v = nn >= 0
            tab2_h[nn[v]] = _bf(o2[v, :OUT])
            tab2_as[nn[v], 0] = o2[v, OUT]
            tab2_ad[nn[v], 0] = o2[v, OUT + 1]
    out = gat_layer(tab2_h, tab2_as, tab2_ad, OUT, 2)
    res = np.zeros((N, OUT), np.float32)
    for c in range(NCORE):
        for t in range(T):
            nn = nid[c, t]
            v = nn >= 0
            res[nn[v]] = out[c, t][v]
    return res


# ---------------------------------------------------------------- bass build
def _build(pre, pay1_bufs=3, pay2_bufs=6):
    import concourse.bacc as bacc
    import concourse.mybir as mybir
    import concourse.tile as tile
    from concourse.masks import make_identity

    d_t = pre["d_t"]
    groups, stripes, goff = pre["groups"], pre["stripes"], pre["goff"]
    CE = int(d_t.sum()) * 8
    MAXS = max(int(goff[g][-1]) for g in range(len(groups)))

    dt = mybir.dt
    nc = bacc.Bacc(num_devices=NCORE, num_swdge_queues=NQ)
    xT = nc.dram_tensor("xT", [IN, NCORE * T * P], dt.bfloat16, kind="ExternalInput")
    W1e = nc.dram_tensor("W1e", [IN, 198], dt.bfloat16, kind="ExternalInput")
    W2e1 = nc.dram_tensor("W2e1", [128, 66], dt.bfloat16, kind="ExternalInput")
    W2e2 = nc.dram_tensor("W2e2", [64, 66], dt.bfloat16, kind="ExternalInput")
    b1i = nc.dram_tensor("b1i", [P, F1], dt.bfloat16, kind="ExternalInput")
    b2b = nc.dram_tensor("b2b", [P, OUT], dt.float32, kind="ExternalInput")
    neg1 = nc.dram_tensor("neg1", [P, F1], dt.bfloat16, kind="ExternalInput")
    sent1 = nc.dram_tensor("sent1", [1, E1], dt.bfloat16, kind="ExternalInput")
    sent2 = nc.dram_tensor("sent2", [1, E2], dt.bfloat16, kind="ExternalInput")
    idx1 = nc.dram_tensor("idx1", [P, CE], dt.int16, kind="ExternalInput")
    idx2 = nc.dram_tensor("idx2", [P, CE], dt.int16, kind="ExternalInput")
    out2 = nc.dram_tensor("out2", [T * P, OUT], dt.float32, kind="ExternalOutput")
    tab1 = nc.dram_tensor("tab1", [NTAB, E1], dt.bfloat16)
    shard = nc.dram_tensor("shard", [CB, E2], dt.bfloat16)
    ag = nc.dram_tensor("ag", [NTAB, E2], dt.bfloat16, addr_space="Shared")

    with tile.TileContext(nc) as tc:
        with tc.tile_pool(name="const", bufs=1) as cp:
            w1_sb = cp.tile([IN, 198], dt.bfloat16)
            nc.sync.dma_start(out=w1_sb[:], in_=W1e[:, :])
            w2a_sb = cp.tile([128, 66], dt.bfloat16)
            nc.sync.dma_start(out=w2a_sb[:], in_=W2e1[:, :])
            w2b_sb = cp.tile([64, 66], dt.bfloat16)
            nc.sync.dma_start(out=w2b_sb[:], in_=W2e2[:, :])
            b1_sb = cp.tile([P, F1], dt.bfloat16)
            nc.sync.dma_start(out=b1_sb[:], in_=b1i[:, :])
            b2_sb = cp.tile([P, OUT], dt.float32)
            nc.sync.dma_start(out=b2_sb[:], in_=b2b[:, :])
            ide = cp.tile([P, P], dt.bfloat16)
            make_identity(nc, ide[:])
            neg1_sb = cp.tile([P, F1], dt.bfloat16)
            nc.sync.dma_start(out=neg1_sb[:], in_=neg1[:, :])
            i1_sb = cp.tile([P, CE], dt.int16)
            nc.sync.dma_start(out=i1_sb[:], in_=idx1[:, :])
            i2_sb = cp.tile([P, CE], dt.int16)
            nc.sync.dma_start(out=i2_sb[:], in_=idx2[:, :])
            sent1_sb = cp.tile([1, E1], dt.bfloat16)
            nc.sync.dma_start(out=sent1_sb[:], in_=sent1[:, :])
            nc.sync.dma_start(out=tab1[SENTROW:SENTROW + 1, :], in_=sent1_sb[:])
            sent2_sb = cp.tile([1, E2], dt.bfloat16)
            nc.sync.dma_start(out=sent2_sb[:], in_=sent2[:, :])
            nc.sync.dma_start(out=shard[T * P:T * P + 1, :], in_=sent2_sb[:])

            # ---------------- dense1: all 50176 rows, rotated layout
            with tc.tile_pool(name="xp", bufs=3) as xp, \
                 tc.tile_pool(name="rowp", bufs=3) as rowp, \
                 tc.tile_pool(name="psD", bufs=4, space="PSUM") as psD:
                for ch in range(NCORE * XC):
                    blk, j = divmod(ch, XC)
                    xch = xp.tile([P, XC * P], dt.bfloat16, tag="x")
                    nc.sync.dma_start(
                        out=xch[:], in_=xT[:, ch * XC * P:(ch + 1) * XC * P])
                    rt = rowp.tile([P, XC * E1], dt.bfloat16, tag="rt")
                    for k in range(XC):
                        pt = psD.tile([P, 198], dt.float32, tag="d1")
                        nc.tensor.matmul(pt[:], lhsT=xch[:, k * P:(k + 1) * P],
                                         rhs=w1_sb[:], start=True, stop=True)
                        if k % 2 == 0:
                            nc.scalar.activation(
                                rt[:, k * E1:k * E1 + F1], pt[:, :F1],
                                mybir.ActivationFunctionType.Copy)
                        else:
                            nc.vector.tensor_copy(
                                out=rt[:, k * E1:k * E1 + F1], in_=pt[:, :F1])
                        nc.vector.tensor_copy(
                            out=rt[:, k * E1 + F1:k * E1 + F1 + 12]
                                .bitcast(dt.float32),
                            in_=pt[:, F1:198])
                    dst = tab1[blk * CB + j * XC * P:
                               blk * CB + (j + 1) * XC * P, :]
                    nc.sync.dma_start(
                        out=dst.rearrange("(k p) e -> p k e", p=P),
                        in_=rt[:].rearrange("p (k e) -> p k e", e=E1))

            # ---------------- L1 edge phase
            tab_lo = tab1[BASE:, :]
            with tc.tile_pool(name="own", bufs=3) as ownp, \
                 tc.tile_pool(name="pay", bufs=pay1_bufs) as payp, \
                 tc.tile_pool(name="wp", bufs=3) as wp, \
                 tc.tile_pool(name="hp", bufs=3) as hp, \
                 tc.tile_pool(name="psU", bufs=3, space="PSUM") as psU, \
                 tc.tile_pool(name="psB", bufs=1, space="PSUM") as psB, \
                 tc.tile_pool(name="psO", bufs=2, space="PSUM") as psO:
                for g, mem in enumerate(groups):
                    M = len(mem)
                    S = int(goff[g][-1])
                    off_cols = int(np.sum([goff[gg][-1] for gg in range(g)]))
                    own = ownp.tile([P, GSZ * E1], dt.bfloat16, tag="own")
                    for mi, t in enumerate(mem):
                        nc.sync.dma_start(
                            out=own[:, mi * E1:(mi + 1) * E1],
                            in_=tab1[t * P:(t + 1) * P, :])
                    pay = payp.tile([P, MAXS * E1], dt.bfloat16, tag="pay")
                    for q, (s0, s1) in enumerate(stripes[g]):
                        nc.gpsimd.dma_gather(
                            out_ap=pay[:, s0 * E1:s1 * E1]
                                .rearrange("p (s e) -> p s e", e=E1),
                            in_ap=tab_lo,
                            idxs_ap=i1_sb[:, (off_cols + s0) * 8:
                                          (off_cols + s1) * 8],
                            num_idxs=(s1 - s0) * P,
                            num_idxs_reg=(s1 - s0) * P,
                            elem_size=E1, single_packet=False, queue_num=q)
                    ownf = own[:].bitcast(dt.float32)
                    ownv = ownf.rearrange("p (m e) -> p m e", e=128)
                    # self logits
                    eS = wp.tile([P, GSZ * 3], dt.float32, tag="eS")
                    nc.vector.tensor_tensor(
                        out=eS[:, :M * 3].rearrange("p (m h) -> p m h", h=3),
                        in0=ownv[:, :M, 96:99], in1=ownv[:, :M, 99:102],
                        op=mybir.AluOpType.add)
                    eS2 = wp.tile([P, GSZ * 3], dt.float32, tag="eS2")
                    nc.vector.scalar_tensor_tensor(
                        out=eS2[:, :M * 3], in0=eS[:, :M * 3], scalar=NEG_SLOPE,
                        in1=eS[:, :M * 3], op0=mybir.AluOpType.mult,
                        op1=mybir.AluOpType.max)
                    wS = wp.tile([P, GSZ * 3], dt.float32, tag="wS")
                    nc.scalar.activation(wS[:, :M * 3], eS2[:, :M * 3],
                                         mybir.ActivationFunctionType.Exp)
                    wSb = wp.tile([P, GSZ * 3], dt.bfloat16, tag="wSb")
                    nc.vector.tensor_copy(out=wSb[:, :M * 3], in_=wS[:, :M * 3])
                    # self multiply in place (interleaved (c,h))
                    oh = own[:].rearrange("p (m e) -> p m e", e=E1)[:, :M, :F1] \
                        .rearrange("p m (c h) -> p m c h", h=3)
                    nc.vector.tensor_tensor(
                        out=oh,
                        in0=oh,
                        in1=wSb[:, :M * 3].rearrange("p (m h) -> p m h", h=3)
                            .unsqueeze(2).to_broadcast([P, M, HID, 3]),
                        op=mybir.AluOpType.mult)
                    # edge logits (per-member add, group-wide rest)
                    et = wp.tile([P, MAXS * 3], dt.float32, tag="et")
                    payf = pay[:].bitcast(dt.float32)
                    a_s = payf.rearrange("p (s e) -> p s e", e=128)
                    for mi, t in enumerate(mem):
                        o0, o1 = int(goff[g][mi]), int(goff[g][mi + 1])
                        nc.vector.tensor_tensor(
                            out=et[:, o0 * 3:o1 * 3]
                                .rearrange("p (s h) -> p s h", h=3),
                            in0=a_s[:, o0:o1, 96:99],
                            in1=ownv[:, mi, 99:102].unsqueeze(1)
                                .to_broadcast([P, o1 - o0, 3]),
                            op=mybir.AluOpType.add)
                    et2 = wp.tile([P, MAXS * 3], dt.float32, tag="et2")
                    nc.vector.scalar_tensor_tensor(
                        out=et2[:, :S * 3], in0=et[:, :S * 3], scalar=NEG_SLOPE,
                        in1=et[:, :S * 3], op0=mybir.AluOpType.mult,
                        op1=mybir.AluOpType.max)
                    wf = wp.tile([P, MAXS * 3], dt.float32, tag="wf")
                    nc.scalar.activation(wf[:, :S * 3], et2[:, :S * 3],
                                         mybir.ActivationFunctionType.Exp)
                    wb = wp.tile([P, MAXS * 3], dt.bfloat16, tag="wb")
                    nc.vector.tensor_copy(out=wb[:, :S * 3], in_=wf[:, :S * 3])
                    # big multiply in place
                    h_view = pay[:].rearrange("p (s e) -> p s e", e=E1)[:, :S, :F1] \
                        .rearrange("p s (c h) -> p s c h", h=3)
                    w_view = wb[:, :S * 3].rearrange("p (s h) -> p s h", h=3) \
                        .unsqueeze(2).to_broadcast([P, S, HID, 3])
                    nc.vector.tensor_tensor(out=h_view, in0=h_view, in1=w_view,
                                            op=mybir.AluOpType.mult)
                    # denominators (per member) + self
                    st = wp.tile([P, GSZ * 3], dt.float32, tag="st")
                    for mi, t in enumerate(mem):
                        o0, o1 = int(goff[g][mi]), int(goff[g][mi + 1])
                        nc.vector.tensor_reduce(
                            out=st[:, mi * 3:(mi + 1) * 3],
                            in_=wf[:, o0 * 3:o1 * 3]
                                .rearrange("p (s h) -> p h s", h=3),
                            axis=mybir.AxisListType.X, op=mybir.AluOpType.add)
                    nc.vector.tensor_add(out=st[:, :M * 3], in0=st[:, :M * 3],
                                         in1=wS[:, :M * 3])
                    nc.vector.tensor_scalar_max(out=st[:, :M * 3],
                                                in0=st[:, :M * 3], scalar1=1e-30)
                    rc = wp.tile([P, GSZ * 3], dt.float32, tag="rc")
                    nc.vector.reciprocal(out=rc[:, :M * 3], in_=st[:, :M * 3])
                    rcb = wp.tile([P, GSZ * 3], dt.bfloat16, tag="rcb")
                    nc.vector.tensor_copy(out=rcb[:, :M * 3], in_=rc[:, :M * 3])
                    # accumulate + psum copy per member
                    h1g = hp.tile([P, GSZ * F1], dt.bfloat16, tag="h1g")
                    for mi, t in enumerate(mem):
                        o0 = int(goff[g][mi])
                        d = int(d_t[t])
                        U = psU.tile([P, F1], dt.float32, tag="U")
                        nc.tensor.matmul(U[:], lhsT=ide[:],
                                         rhs=own[:, mi * E1:mi * E1 + F1],
                                         start=True, stop=False)
                        for s in range(d):
                            nc.tensor.matmul(
                                U[:], lhsT=ide[:],
                                rhs=pay[:, (o0 + s) * E1:(o0 + s) * E1 + F1],
                                start=False, stop=(s == d - 1))
                        nc.scalar.activation(h1g[:, mi * F1:(mi + 1) * F1],
                                             U[:],
                                             mybir.ActivationFunctionType.Copy)
                    # group-wide normalize + bias + ELU
                    h1v = h1g[:, :M * F1].rearrange("p (m c h) -> p m c h", h=3, c=HID)
                    nc.vector.tensor_tensor(
                        out=h1v, in0=h1v,
                        in1=rcb[:, :M * 3].rearrange("p (m h) -> p m h", h=3)
                            .unsqueeze(2).to_broadcast([P, M, HID, 3]),
                        op=mybir.AluOpType.mult)
                    nc.vector.tensor_tensor(
                        out=h1g[:, :M * F1].rearrange("p (m f) -> p m f", f=F1),
                        in0=h1g[:, :M * F1].rearrange("p (m f) -> p m f", f=F1),
                        in1=b1_sb[:].unsqueeze(1).to_broadcast([P, M, F1]),
                        op=mybir.AluOpType.add)
                    tmin = hp.tile([P, GSZ * F1], dt.bfloat16, tag="tmin")
                    nc.vector.scalar_tensor_tensor(
                        out=tmin[:, :M * F1], in0=h1g[:, :M * F1], scalar=0.0,
                        in1=h1g[:, :M * F1], op0=mybir.AluOpType.mult,
                        op1=mybir.AluOpType.min)
                    texp = hp.tile([P, GSZ * F1], dt.bfloat16, tag="texp")
                    nc.scalar.activation(texp[:, :M * F1], tmin[:, :M * F1],
                                         mybir.ActivationFunctionType.Exp)
                    nc.vector.scalar_tensor_tensor(
                        out=h1g[:, :M * F1], in0=h1g[:, :M * F1], scalar=0.0,
                        in1=texp[:, :M * F1], op0=mybir.AluOpType.max,
                        op1=mybir.AluOpType.add)
                    nc.vector.tensor_tensor(
                        out=h1g[:, :M * F1].rearrange("p (m f) -> p m f", f=F1),
                        in0=h1g[:, :M * F1].rearrange("p (m f) -> p m f", f=F1),
                        in1=neg1_sb[:].unsqueeze(1).to_broadcast([P, M, F1]),
                        op=mybir.AluOpType.add)
                    # dense2 per member
                    for mi, t in enumerate(mem):
                        tp1 = psB.tile([P, P], dt.bfloat16, tag="tp1")
                        nc.tensor.transpose(tp1[:], h1g[:, mi * F1:mi * F1 + P],
                                            ide[:])
                        tp2 = psB.tile([64, P], dt.bfloat16, tag="tp2")
                        nc.tensor.transpose(tp2[:],
                                            h1g[:, mi * F1 + P:(mi + 1) * F1],
                                            ide[:])
                        hT1 = hp.tile([P, P], dt.bfloat16, tag="hT1")
                        nc.vector.tensor_copy(out=hT1[:], in_=tp1[:])
                        hT2 = hp.tile([64, P], dt.bfloat16, tag="hT2")
                        nc.vector.tensor_copy(out=hT2[:], in_=tp2[:])
                        o2 = psO.tile([P, 66], dt.float32, tag="o2")
                        nc.tensor.matmul(o2[:], lhsT=hT1[:], rhs=w2a_sb[:],
                                         start=True, stop=False)
                        nc.tensor.matmul(o2[:], lhsT=hT2[:], rhs=w2b_sb[:],
                                         start=False, stop=True)
                        r2 = hp.tile([P, 68], dt.bfloat16, tag="r2")
                        nc.scalar.activation(r2[:, :OUT], o2[:, :OUT],
                                             mybir.ActivationFunctionType.Copy)
                        nc.vector.tensor_copy(
                            out=r2[:, OUT:OUT + 4].bitcast(dt.float32),
                            in_=o2[:, OUT:OUT + 2])
                        nc.sync.dma_start(out=shard[t * P:(t + 1) * P, :68],
                                          in_=r2[:])

            # ---------------- AllGather table2 (two halves)
            nc.gpsimd.collective_compute(
                "AllGather", mybir.AluOpType.bypass,
                replica_groups=[list(range(NCORE))],
                ins=[shard[0:HALF_T * P, :].opt()],
                outs=[ag[0:NCORE * HALF_T * P, :].opt()])
            nc.gpsimd.collective_compute(
                "AllGather", mybir.AluOpType.bypass,
                replica_groups=[list(range(NCORE))],
                ins=[shard[HALF_T * P:CB, :].opt()],
                outs=[ag[NCORE * HALF_T * P:NTAB, :].opt()])

            # ---------------- L2 edge phase
            ag_lo = ag[BASE:, :]
            with tc.tile_pool(name="own2", bufs=3) as ownp2, \
                 tc.tile_pool(name="pay2", bufs=pay2_bufs) as payp2, \
                 tc.tile_pool(name="wp2", bufs=3) as wp2, \
                 tc.tile_pool(name="op2", bufs=3) as op2, \
                 tc.tile_pool(name="psU2", bufs=4, space="PSUM") as psU2:
                for g, mem in enumerate(groups):
                    M = len(mem)
                    S = int(goff[g][-1])
                    off_cols = int(np.sum([goff[gg][-1] for gg in range(g)]))
                    own = ownp2.tile([P, GSZ * E2], dt.bfloat16, tag="own")
                    for mi, t in enumerate(mem):
                        nc.sync.dma_start(
                            out=own[:, mi * E2:(mi + 1) * E2],
                            in_=shard[t * P:(t + 1) * P, :])
                    pay = payp2.tile([P, MAXS * E2], dt.bfloat16, tag="pay")
                    for q, (s0, s1) in enumerate(stripes[g]):
                        nc.gpsimd.dma_gather(
                            out_ap=pay[:, s0 * E2:s1 * E2]
                                .rearrange("p (s e) -> p s e", e=E2),
                            in_ap=ag_lo,
                            idxs_ap=i2_sb[:, (off_cols + s0) * 8:
                                          (off_cols + s1) * 8],
                            num_idxs=(s1 - s0) * P,
                            num_idxs_reg=(s1 - s0) * P,
                            elem_size=E2, single_packet=False, queue_num=q)
                    ownf = own[:].bitcast(dt.float32)
                    ownv = ownf.rearrange("p (m e) -> p m e", e=64)
                    eS = wp2.tile([P, GSZ], dt.float32, tag="eS")
                    nc.vector.tensor_tensor(
                        out=eS[:, :M].unsqueeze(2),
                        in0=ownv[:, :M, 32:33], in1=ownv[:, :M, 33:34],
                        op=mybir.AluOpType.add)
                    eS2 = wp2.tile([P, GSZ], dt.float32, tag="eS2")
                    nc.vector.scalar_tensor_tensor(
                        out=eS2[:, :M], in0=eS[:, :M], scalar=NEG_SLOPE,
                        in1=eS[:, :M], op0=mybir.AluOpType.mult,
                        op1=mybir.AluOpType.max)
                    wS = wp2.tile([P, GSZ], dt.float32, tag="wS")
                    nc.scalar.activation(wS[:, :M], eS2[:, :M],
                                         mybir.ActivationFunctionType.Exp)
                    wSb = wp2.tile([P, GSZ], dt.bfloat16, tag="wSb")
                    nc.vector.tensor_copy(out=wSb[:, :M], in_=wS[:, :M])
                    oh = own[:].rearrange("p (m e) -> p m e", e=E2)[:, :M, :OUT]
                    nc.vector.tensor_tensor(
                        out=oh, in0=oh,
                        in1=wSb[:, :M].unsqueeze(2).to_broadcast([P, M, OUT]),
                        op=mybir.AluOpType.mult)
                    et = wp2.tile([P, MAXS], dt.float32, tag="et")
                    payf = pay[:].bitcast(dt.float32)
                    a_s = payf.rearrange("p (s e) -> p s e", e=64)
                    for mi, t in enumerate(mem):
                        o0, o1 = int(goff[g][mi]), int(goff[g][mi + 1])
                        nc.vector.tensor_tensor(
                            out=et[:, o0:o1].unsqueeze(2),
                            in0=a_s[:, o0:o1, 32:33],
                            in1=ownv[:, mi, 33:34].unsqueeze(1)
                                .to_broadcast([P, o1 - o0, 1]),
                            op=mybir.AluOpType.add)
                    et2 = wp2.tile([P, MAXS], dt.float32, tag="et2")
                    nc.vector.scalar_tensor_tensor(
                        out=et2[:, :S], in0=et[:, :S], scalar=NEG_SLOPE,
                        in1=et[:, :S], op0=mybir.AluOpType.mult,
                        op1=mybir.AluOpType.max)
                    wf = wp2.tile([P, MAXS], dt.float32, tag="wf")
                    nc.scalar.activation(wf[:, :S], et2[:, :S],
                                         mybir.ActivationFunctionType.Exp)
                    wb = wp2.tile([P, MAXS], dt.bfloat16, tag="wb")
                    nc.vector.tensor_copy(out=wb[:, :S], in_=wf[:, :S])
                    h_view = pay[:].rearrange("p (s e) -> p s e", e=E2)[:, :S, :OUT]
                    w_view = wb[:, :S].unsqueeze(2).to_broadcast([P, S, OUT])
                    nc.vector.tensor_tensor(out=h_view, in0=h_view, in1=w_view,
                                            op=mybir.AluOpType.mult)
                    st = wp2.tile([P, GSZ], dt.float32, tag="st")
                    for mi, t in enumerate(mem):
                        o0, o1 = int(goff[g][mi]), int(goff[g][mi + 1])
                        nc.vector.tensor_reduce(
                            out=st[:, mi:mi + 1], in_=wf[:, o0:o1],
                            axis=mybir.AxisListType.X, op=mybir.AluOpType.add)
                    nc.vector.tensor_add(out=st[:, :M], in0=st[:, :M],
                                         in1=wS[:, :M])
                    nc.vector.tensor_scalar_max(out=st[:, :M], in0=st[:, :M],
                                                scalar1=1e-30)
                    rc = wp2.tile([P, GSZ], dt.float32, tag="rc")
                    nc.vector.reciprocal(out=rc[:, :M], in_=st[:, :M])
                    for mi, t in enumerate(mem):
                        o0 = int(goff[g][mi])
                        d = int(d_t[t])
                        U = psU2.tile([P, OUT], dt.float32, tag="U2")
                        nc.tensor.matmul(U[:], lhsT=ide[:],
                                         rhs=own[:, mi * E2:mi * E2 + OUT],
                                         start=True, stop=False)
                        for s in range(d):
                            nc.tensor.matmul(
                                U[:], lhsT=ide[:],
                                rhs=pay[:, (o0 + s) * E2:(o0 + s) * E2 + OUT],
                                start=False, stop=(s == d - 1))
                        ot = op2.tile([P, OUT], dt.float32, tag="ot")
                        nc.scalar.activation(ot[:], U[:],
                                             mybir.ActivationFunctionType.Copy)
                        nc.vector.tensor_tensor(
                            out=ot[:], in0=ot[:],
                            in1=rc[:, mi:mi + 1].to_broadcast([P, OUT]),
                            op=mybir.AluOpType.mult)
                        nc.vector.tensor_add(out=ot[:], in0=ot[:], in1=b2_sb[:])
                        nc.sync.dma_start(out=out2[t * P:(t + 1) * P, :],
                                          in_=ot[:])
    nc.compile()
    return nc


# ---------------------------------------------------------------- kernel
def kernel(x, edge_index, W1, att_src1, att_dst1, b1, W2, att_src2, att_dst2,
           b2, _emulate=False, _timing=None):
    x = np.asarray(x, np.float32)
    edge_index = np.asarray(edge_index)
    W1 = np.asarray(W1, np.float32)
    att_src1 = np.asarray(att_src1, np.float32)
    att_dst1 = np.asarray(att_dst1, np.float32)
    b1 = np.asarray(b1, np.float32)
    W2 = np.asarray(W2, np.float32)
    att_src2 = np.asarray(att_src2, np.float32)
    att_dst2 = np.asarray(att_dst2, np.float32)
    b2 = np.asarray(b2, np.float32)

    if _emulate:
        return emulate(x, edge_index, W1, att_src1, att_dst1, b1,
                       W2, att_src2, att_dst2, b2)

    from concourse.bass_utils import run_bass_kernel_spmd
    import time as _time

    pre = preprocess(edge_index)
    hw = host_weights(x, W1, att_src1, att_dst1, b1, W2, att_src2, att_dst2,
                      b2, pre["nid"])
    nc = _build(pre)
    neg1 = np.full((P, F1), -1.0, bf16)
    maps = [dict(xT=hw["xTs"][c], W1e=hw["W1e"], W2e1=hw["W2e1"],
                 W2e2=hw["W2e2"], b1i=hw["b1i"], b2b=hw["b2b"],
                 neg1=neg1, sent1=hw["sent1"], sent2=hw["sent2"],
                 idx1=pre["idx1"][c], idx2=pre["idx2"][c])
            for c in range(NCORE)]

    trace = _timing is not None
    res = None
    for attempt in range(3):
        try:
            res = run_bass_kernel_spmd(nc, maps, core_ids=list(range(NCORE)),
                                       trace=trace and attempt == 0)
            break
        except Exception:
            if attempt == 2:
                raise
            _time.sleep(45)

    nid = pre["nid"]
    out = np.zeros((N, OUT), np.float32)
    for c in range(NCORE):
        o = res.results[c]["out2"]
        nn = nid[c].reshape(-1)
        valid = nn >= 0
        out[nn[valid]] = o[valid]

    if _timing is not None:
        _timing["neff1_ns"] = res.exec_time_ns
        _timing["neff2_ns"] = 0
    return out
